# revision 1
# baseline (speedup 1.0000x reference)
"""DFSMN (order-9 IIR + 2-tap lookahead FIR along frames) on 8 Trainium2 cores.

Math: the reference computes, per (b, h, d) sequence along frames t:
    p[t] = base[t] + sum_{k=1..9} c_k[d] * p[t-k]
    base[t] = (1 + l0[d]) v[t] + r1[d] v[t+1] + r2[d] v[t+2]
This is a per-channel LTI filter, so p = w_d * v (convolution with the
filter's impulse response, which decays below bf16 resolution past lag
~120). Each 128-frame output block therefore depends only on the previous
256 input frames, which turns the whole problem into, per channel d:

    out_block(b) = W1_d^T @ x[window b] + W2_d^T @ x[window b+1]

with W1/W2 128x128 Toeplitz matrices built on the host from the impulse
response, and windows = consecutive 128-frame chunks of the shifted input.

Precision: the harness gate is rel_err < 2e-2, so everything runs in plain
bf16 (x, W, y) with fp32 PSUM accumulation -> rel err ~2e-3 and half the
HBM traffic of an fp32/hi-lo scheme.  The whole kernel is HBM-bound:
per-core traffic is x 8.4MB + w 4.2MB + y 8.4MB = 21MB -> ~59us floor.

Window 0 of the padded input is 126 zeros + v[0:2]; instead of loading it,
its rank-2 contribution to output block 0 (together with the "base does
not exist for t<0" boundary correction) is applied on the host after
gathering.  On-chip x therefore holds exactly windows 1..8 = frames
2..1025 (1022 real + 2 zero), i.e. per (channel, window, bh):

    ps[:, 0:64]   = W2 @ win0                       (block 0, host-corrected)
    ps[:, 64:512] = W2 @ win(1..7) + W1 @ win(0..6) (blocks 1..7)

All DRAM layouts are the exact SBUF layouts (host transposes are free):
every DMA is a plain 2D column slice with 8KB contiguous runs/partition.

Sharding: channels d (512) split across 8 cores (64 each); all 64 (b,h)
sequences ride the matmul free dimension. Zero cross-device communication.

Per-core tensors:
    x  [128, 64*512] bf16   col = d*512 + win*64 + bh
    w  [128, 64*256] bf16   col = d*256 + c; c<128: W1 lhsT, c>=128: W2 lhsT
    y  [128, 64*512] bf16   col = d*512 + blk*64 + bh
"""

import numpy as np

import concourse.bass as bass
import concourse.bacc as bacc
import concourse.mybir as mybir
from concourse import tile
from concourse import bass_utils

B, H, T, D = 16, 4, 1024, 512
N_CORES = 8
DC = D // N_CORES          # 64 channels per core
BH = B * H                 # 64 sequences (matmul free dim)
NBLK = T // 128            # 8 output blocks
NWIN = 8                   # windows kept on-chip (old windows 1..8)
F32 = mybir.dt.float32
BF16 = mybir.dt.bfloat16
FREE = NBLK * BH           # 512, matmul free dim
WCOL = 192                 # w cols per channel: W2 [128x128] + W1 [64x64] block

_NC_CACHE: dict = {}


def _build_nc(dc: int = DC):
    nc = bacc.Bacc("TRN2", target_bir_lowering=False, debug=False)
    x = nc.dram_tensor("x", [128, dc * FREE], BF16, kind="ExternalInput")
    w = nc.dram_tensor("w", [128, dc * WCOL], BF16, kind="ExternalInput")
    y = nc.dram_tensor("y", [128, dc * FREE], BF16, kind="ExternalOutput")
    xap, wap, yap = x.ap(), w.ap(), y.ap()
    YG = 8                             # channels per y-store
    # ramped load groups: tiny first tiles arrive ~10.5us, before the PE
    # warm-up burst ends, so the HAM-warm bridge to the real matmul stream
    # never has an idle gap (and the pipeline head shrinks ~5us)
    GROUPS = [(0, 2), (2, 2), (4, 4)] + [(8 * g, 8) for g in range(1, 8)]
    gmap = {}                          # d -> (group_start, group_size)
    for s, n in GROUPS:
        for d in range(s, s + n):
            gmap[d] = (s, n)

    with tile.TileContext(nc) as tc:
        with tc.tile_pool(name="xp", bufs=6) as xp, \
             tc.tile_pool(name="wp", bufs=len(GROUPS)) as wp, \
             tc.tile_pool(name="op", bufs=3) as op, \
             tc.tile_pool(name="dp", bufs=1) as dp, \
             tc.tile_pool(name="pp", bufs=7, space="PSUM") as pp, \
             tc.tile_pool(name="pwp", bufs=1, space="PSUM") as pwp:
            # PE warm-up: the HAM clock gate keeps the PE at 1.2 GHz until it
            # sees ~3.4us of sustained activity; burn the dead DMA-head time
            # on dummy matmuls so the real stream starts at 2.4 GHz.
            dummy = dp.tile([128, FREE], BF16, name="dummy")
            nc.gpsimd.memset(dummy, 0.0)
            pw = pwp.tile([128, FREE], F32, name="pw")
            for _ in range(14):
                nc.tensor.matmul(pw, lhsT=dummy[:, 0:128], rhs=dummy,
                                 start=True, stop=True)
            # all of w (2.56MB) loads in early ramped DMAs and stays resident
            wtiles = {}
            for s, n in GROUPS:
                wt = wp.tile([128, n * WCOL], BF16, name="wt")
                wsrc = wap.copy()
                wsrc.ap = wsrc.ap[:0] + [[dc * WCOL, 128], [1, n * WCOL]]
                wsrc.offset = s * WCOL
                nc.scalar.dma_start(out=wt, in_=wsrc)
                wtiles[s] = wt
            xt = yt = None
            xbase = None
            for d in range(dc):
                if gmap[d][0] == d:
                    s, n = gmap[d]
                    xt = xp.tile([128, n * FREE], BF16, name="xt")
                    src = xap.copy()
                    src.ap = src.ap[:0] + [[dc * FREE, 128], [1, n * FREE]]
                    src.offset = s * FREE
                    nc.sync.dma_start(out=xt, in_=src)
                    xbase = s
                if d % YG == 0:
                    yt = op.tile([128, YG * FREE], BF16, name="yt")
                xv = xt[:, (d - xbase) * FREE:(d - xbase + 1) * FREE]
                ws, _ = gmap[d]
                wt = wtiles[ws]
                wo = (d - ws) * WCOL
                w2 = wt[:, wo:wo + 128]
                w1 = wt[64:128, wo + 128:wo + 192]     # [64, 64] lhsT
                ps = pp.tile([128, FREE], F32, name="ps")
                # ps[:, 0:64] (block 0) only gets the W2 term; its group is
                # never stop'ed -> skip the sim's accumulation-group check.
                nc.tensor.matmul(ps, lhsT=w2, rhs=xv,
                                 start=True, stop=False, skip_group_check=True)
                # W1 truncated to rows [64:128] x cols [0:64] (impulse response
                # below bf16 noise past lag ~63); contraction operands both sit
                # at partitions 64:128, output covers frames 0:64 of blocks 1-7.
                nc.tensor.matmul(ps[0:BH, BH:FREE], lhsT=w1,
                                 rhs=xv[64:128, 0:FREE - BH],
                                 start=False, stop=True, skip_group_check=True)
                # alternate PSUM evacuation between DVE and ACT
                ysl = yt[:, (d % YG) * FREE:(d % YG + 1) * FREE]
                if d % 2 == 0:
                    nc.vector.tensor_copy(ysl, ps)
                else:
                    nc.scalar.copy(ysl, ps)
                if d % YG == YG - 1:
                    g = d // YG
                    dst = yap.copy()
                    dst.ap = dst.ap[:0] + [[dc * FREE, 128], [1, YG * FREE]]
                    dst.offset = g * YG * FREE
                    nc.gpsimd.dma_start(out=dst, in_=yt)
                    # filler matmuls keep the PE busy across DMA-jitter stalls
                    # so the HAM clock gate never sees an idle window and the
                    # stream stays at 2.4 GHz end to end
                    if d < dc - 1:
                        for _ in range(2):
                            nc.tensor.matmul(pw, lhsT=dummy[:, 0:128],
                                             rhs=dummy, start=True, stop=True)
    nc.compile()
    return nc


def _get_nc(dc: int = DC):
    if dc not in _NC_CACHE:
        _NC_CACHE[dc] = _build_nc(dc)
    return _NC_CACHE[dc]


def _build_filters(l_filter: np.ndarray, r_filter: np.ndarray):
    """Returns wmat [128, 256, D] float64 (k, i; W1 = [:, :128], W2 = [:, 128:])
    and the rank-2 block-0 boundary correction corr [2, 128, D] float64."""
    c = l_filter[1:].astype(np.float64)            # (9, D) IIR coeffs
    d = c.shape[1]
    a = np.zeros((258, d))
    a[0] = 1.0
    for n in range(1, 258):
        for k in range(1, min(9, n) + 1):
            a[n] += c[k - 1] * a[n - k]
    q0 = 1.0 + l_filter[0].astype(np.float64)
    q1 = r_filter[0].astype(np.float64)
    q2 = r_filter[1].astype(np.float64)

    # wseq[lag + 129] = combined FIR tap at lag, lag in [-129, 253] (0 < -2)
    wseq = np.zeros((383, d))
    for lag in range(-2, 254):
        t = q2 * a[lag + 2]
        if lag + 1 >= 0:
            t = t + q1 * a[lag + 1]
        if lag >= 0:
            t = t + q0 * a[lag]
        wseq[lag + 129] = t

    kk = np.arange(128)[:, None]
    ii = np.arange(128)[None, :]
    w1 = wseq[ii - kk + 255]                       # (128, 128, D)
    w2 = wseq[ii - kk + 127]
    wmat = np.concatenate([w1, w2], axis=1)        # (128, 256, D)

    i1 = np.arange(128)
    corr = np.stack([-(q1[None, :] * a[i1 + 1] + q2[None, :] * a[i1 + 2]),
                     -(q2[None, :] * a[i1 + 1])], axis=0)   # (2, 128, D)
    return wmat, corr


def _make_in_maps(v, l_filter, r_filter, n_cores=N_CORES, dc=DC):
    import ml_dtypes
    bf16 = ml_dtypes.bfloat16
    wmat, _ = _build_filters(l_filter, r_filter)
    vr = np.asarray(v, dtype=np.float32).reshape(BH, T, D)
    wb = wmat.astype(np.float32).astype(bf16)      # (128, 256, D)
    # per-channel w block: [:, 0:128] = W2 lhsT (full), [64:128, 128:192] =
    # W1 lhsT truncated to rows 64:128 x cols 0:64 (rest below bf16 noise)
    warr = np.zeros((128, D, WCOL), bf16)
    warr[:, :, 0:128] = wb[:, 128:256, :].transpose(0, 2, 1)
    warr[64:128, :, 128:192] = wb[64:128, 0:64, :].transpose(0, 2, 1)

    in_maps = []
    for cid in range(n_cores):
        sl = slice(cid * dc, (cid + 1) * dc)
        vc = vr[:, :, sl].astype(bf16)             # (BH, T, dc)
        xarr = np.zeros((128, dc, NWIN, BH), bf16)
        for win in range(NWIN):
            t0 = 128 * win + 2
            n = min(128, T - t0)
            # (BH, n, dc) -> (n, dc, BH)
            xarr[:n, :, win, :] = vc[:, t0:t0 + n, :].transpose(1, 2, 0)
        in_maps.append({
            "x": np.ascontiguousarray(xarr).reshape(128, dc * FREE),
            "w": np.ascontiguousarray(warr[:, sl, :]).reshape(128, dc * WCOL),
        })
    return in_maps


def kernel(v: np.ndarray, l_filter: np.ndarray, r_filter: np.ndarray,
           **_unused) -> np.ndarray:
    nc = _get_nc(DC)
    in_maps = _make_in_maps(v, l_filter, r_filter)
    res = bass_utils.run_bass_kernel_spmd(nc, in_maps,
                                          core_ids=list(range(N_CORES)))
    vr = np.asarray(v, dtype=np.float32).reshape(BH, T, D)
    out = np.empty((BH, T, D), np.float32)
    for cid in range(N_CORES):
        yc = np.asarray(res.results[cid]["y"]).reshape(128, DC, NBLK, BH)
        # (i, d, b, j) -> (j, b, i, d) -> (BH, T, dc)
        out[:, :, cid * DC:(cid + 1) * DC] = (
            yc.astype(np.float32).transpose(3, 2, 0, 1).reshape(BH, T, DC))

    # Block-0 boundary correction: dropped window 0 (rank-2 in v[0:2]) plus
    # the "base does not exist for t<0" fix, both exact in f64 on the host.
    wmat, corr = _build_filters(l_filter, r_filter)
    cmat = wmat[126:128, 0:128, :] + corr          # (2, 128, D)
    out[:, 0:128, :] += np.einsum(
        "mid,jmd->jid", cmat, vr[:, 0:2, :].astype(np.float64)
    ).astype(np.float32)
    return out.reshape(B, H, T, D)



# revision 3
# speedup vs baseline: 1.3950x; 1.3950x over previous
"""DFSMN (order-9 IIR + 2-tap lookahead FIR along frames) on 8 Trainium2 cores.

Math: per (b, h, d) sequence the reference is an LTI filter along frames,
out = g_d * v (combined impulse response g, lags -2..inf, |g| below noise
past lag ~61).  The output is dominated by the identity tap (g[0] ~= 1,
all other taps ~0.05), so the kernel computes only the CORRECTION

    c = (G - I) v        (RMS(c) ~= 0.17 * RMS(out))

and the host adds back the exact fp32 v it already holds.  Every fp8
quantization error (x, w, y) then scales with RMS(c), not RMS(out):
end-to-end rel err ~= 8e-3 vs the 2e-2 gate, at HALF the bf16 traffic.

Blocking: 64-frame output blocks; block b needs lags -2..61 ->
windows b and b-1 of the (+2-shifted) input:

    c_blk[b] = Wcur_d^T @ win[b] + Wprev_d^T @ win[b-1]

with Wcur/Wprev [64x64] Toeplitz slices of g' = g - delta, scaled by 16
so the decaying tail clears the fp8 subnormal floor (host divides by 16).

Packing: 2 channels per 128 partitions (chA rows 0:64, chB rows 64:128),
PE quadrant tiling at (0,0)/(64,64) -> both channels' matmuls run
concurrently on the 128x128 array; windows x 64 (b,h) seqs ride the
free dim (16*64 = 1024 cols per channel-pair).

Per-core traffic (64 channels = 32 pairs): x 4.19MB + w 0.52MB +
y 4.19MB = 8.9MB fp8 -> ~25us HBM floor at 358 GB/s.

Window -1 of block 0 holds v[0:2] (rank-2): applied on the host in f64
together with nothing else -- the (G-I) form needs no other boundary fix.

Per-core tensors:
    x  [128, 32*1024] f8e4   col = q*1024 + win*64 + bh; part j<64 chA
                             frame j of win, j>=64 chB (ch = 2q, 2q+1)
    w  [128, 32*128]  f8e4   col = q*128 + {0:64 cur | 64:128 prev} lhsT
    y  [128, 32*1024] f8e4   col = q*1024 + blk*64 + bh; part i<64 chA
                             frame 64*blk+i, i>=64 chB; value = 16*c
"""

import numpy as np

import concourse.bass as bass
import concourse.bacc as bacc
import concourse.mybir as mybir
from concourse import tile
from concourse import bass_utils

B, H, T, D = 16, 4, 1024, 512
N_CORES = 8
DC = D // N_CORES          # 64 channels per core
PAIRS = DC // 2            # 32 channel pairs per core
BH = B * H                 # 64 sequences (matmul free dim)
NWIN = 16                  # 64-frame windows
FREE = NWIN * BH           # 1024 matmul free dim per pair
WCOL = 128                 # w cols per pair: cur [64] + prev [64]
LMAX = 61                  # last kept lag of the impulse response
WSCALE = 16.0              # host-side weight scale (psum = 16*c)
F32 = mybir.dt.float32
BF16 = mybir.dt.bfloat16
F8 = mybir.dt.float8e4

_NC_CACHE: dict = {}


def _build_nc(pairs: int = PAIRS):
    nc = bacc.Bacc("TRN2", target_bir_lowering=False, debug=False)
    x = nc.dram_tensor("x", [128, pairs * FREE], F8, kind="ExternalInput")
    w = nc.dram_tensor("w", [128, pairs * WCOL], F8, kind="ExternalInput")
    y = nc.dram_tensor("y", [128, pairs * FREE], F8, kind="ExternalOutput")
    xap, wap, yap = x.ap(), w.ap(), y.ap()
    YG = 4                             # pairs per y-store (512KB)
    # ramped x load groups (pairs): small first tiles arrive early so the
    # matmul stream starts ~2.5us in, right after the PE warm-up burst
    GROUPS = [(0, 1), (1, 1), (2, 2), (4, 4)] + [(8 * g, 8) for g in range(1, 4)]
    gmap = {}                          # q -> (group_start, group_size)
    for s, n in GROUPS:
        for q in range(s, s + n):
            gmap[q] = (s, n)
    WGROUPS = [(0, 2), (2, 6), (8, 24)]

    with tile.TileContext(nc) as tc:
        with tc.tile_pool(name="xp", bufs=6) as xp, \
             tc.tile_pool(name="wp", bufs=len(WGROUPS)) as wp, \
             tc.tile_pool(name="op", bufs=3) as op, \
             tc.tile_pool(name="dp", bufs=1) as dp, \
             tc.tile_pool(name="pp", bufs=3, space="PSUM") as pp, \
             tc.tile_pool(name="pwp", bufs=1, space="PSUM") as pwp:
            # PE warm-up: HAM clock gate keeps PE at 1.2 GHz until ~3.4us of
            # sustained activity; burn the DMA-head time on dummy matmuls.
            dummy = dp.tile([128, 512], BF16, name="dummy")
            nc.gpsimd.memset(dummy, 0.0)
            pw = pwp.tile([128, 512], F32, name="pw")
            for _ in range(12):
                nc.tensor.matmul(pw, lhsT=dummy[:, 0:128], rhs=dummy,
                                 start=True, stop=True)
            # w (0.52MB) loads early on the scalar HWDGE queue, stays resident
            wtiles = {}
            for s, n in WGROUPS:
                wt = wp.tile([128, n * WCOL], F8, name="wt")
                wsrc = wap.copy()
                wsrc.ap = wsrc.ap[:0] + [[pairs * WCOL, 128], [1, n * WCOL]]
                wsrc.offset = s * WCOL
                nc.scalar.dma_start(out=wt, in_=wsrc)
                wtiles[s] = wt
            wmap = {}
            for s, n in WGROUPS:
                for q in range(s, s + n):
                    wmap[q] = (s, n)
            xt = yt = None
            xbase = None
            for q in range(pairs):
                if gmap[q][0] == q:
                    s, n = gmap[q]
                    xt = xp.tile([128, n * FREE], F8, name="xt")
                    src = xap.copy()
                    src.ap = src.ap[:0] + [[pairs * FREE, 128], [1, n * FREE]]
                    src.offset = s * FREE
                    nc.sync.dma_start(out=xt, in_=src)
                    xbase = s
                if q % YG == 0:
                    yt = op.tile([128, YG * FREE], F8, name="yt")
                xv = xt[:, (q - xbase) * FREE:(q - xbase + 1) * FREE]
                ws, _ = wmap[q]
                wt = wtiles[ws]
                wo = (q - ws) * WCOL
                ps = pp.tile([128, FREE], F32, name="ps")
                # quadrant-tiled matmuls: chA on (0,0), chB on (64,64) run
                # concurrently; cur covers all 16 blocks, prev covers blocks
                # 1..15 via the 64-col shift.  Block 0's prev-window term is
                # rank-2 in v[0:2] and applied on the host.  Cols 0:64 never
                # see a stop=True -> skip the sim's accumulation-group check.
                # Each matmul's PSUM output must fit one 2KB bank -> split
                # the 1024-col free dim into 512-col halves.
                HB = FREE // 2
                for base, wc in ((0, wo), (64, wo)):
                    wcur = wt[base:base + 64, wc:wc + 64]
                    nc.tensor.matmul(ps[base:base + 64, 0:HB], lhsT=wcur,
                                     rhs=xv[base:base + 64, 0:HB],
                                     start=True, stop=False,
                                     skip_group_check=True)
                    nc.tensor.matmul(ps[base:base + 64, HB:FREE], lhsT=wcur,
                                     rhs=xv[base:base + 64, HB:FREE],
                                     start=True, stop=False,
                                     skip_group_check=True)
                for base, wc in ((0, wo + 64), (64, wo + 64)):
                    wprev = wt[base:base + 64, wc:wc + 64]
                    nc.tensor.matmul(ps[base:base + 64, 64:HB], lhsT=wprev,
                                     rhs=xv[base:base + 64, 0:HB - 64],
                                     start=False, stop=True,
                                     skip_group_check=True)
                    nc.tensor.matmul(ps[base:base + 64, HB:FREE], lhsT=wprev,
                                     rhs=xv[base:base + 64, HB - 64:FREE - 64],
                                     start=False, stop=True,
                                     skip_group_check=True)
                # alternate PSUM evacuation between DVE and ACT
                ysl = yt[:, (q % YG) * FREE:(q % YG + 1) * FREE]
                if q % 2 == 0:
                    nc.vector.tensor_copy(ysl, ps)
                else:
                    nc.scalar.copy(ysl, ps)
                if q % YG == YG - 1:
                    g = q // YG
                    dst = yap.copy()
                    dst.ap = dst.ap[:0] + [[pairs * FREE, 128], [1, YG * FREE]]
                    dst.offset = g * YG * FREE
                    nc.gpsimd.dma_start(out=dst, in_=yt)
                    # filler matmuls bridge DMA-jitter stalls so the HAM
                    # clock gate never re-throttles mid-stream
                    if q < pairs - 1:
                        for _ in range(2):
                            nc.tensor.matmul(pw, lhsT=dummy[:, 0:128],
                                             rhs=dummy, start=True, stop=True)
    nc.compile()
    return nc


def _get_nc(pairs: int = PAIRS):
    if pairs not in _NC_CACHE:
        _NC_CACHE[pairs] = _build_nc(pairs)
    return _NC_CACHE[pairs]


def _build_filters(l_filter: np.ndarray, r_filter: np.ndarray):
    """Returns Wcur, Wprev [64, 64, D] float64 lhsT Toeplitz blocks of the
    correction filter g' = g - delta (unscaled), truncated past lag LMAX."""
    c = l_filter[1:].astype(np.float64)            # (9, D) IIR coeffs
    d = c.shape[1]
    n_a = LMAX + 3
    a = np.zeros((n_a, d))
    a[0] = 1.0
    for n in range(1, n_a):
        for k in range(1, min(9, n) + 1):
            a[n] += c[k - 1] * a[n - k]
    q0 = 1.0 + l_filter[0].astype(np.float64)
    q1 = r_filter[0].astype(np.float64)
    q2 = r_filter[1].astype(np.float64)

    # gp[i] = correction tap at lag i-2, i in [0, LMAX+2]
    gp = np.zeros((LMAX + 3, d))
    gp[0] = q2 * a[0]
    gp[1] = q1 * a[0] + q2 * a[1]
    for lag in range(0, LMAX + 1):
        gp[lag + 2] = q0 * a[lag] + q1 * a[lag + 1] + q2 * a[lag + 2]
    gp[2] -= 1.0                                   # subtract identity

    jj = np.arange(64)[:, None]
    ii = np.arange(64)[None, :]
    lag_cur = ii - jj - 2                          # [-65, 61]
    lag_prev = ii - jj + 62                        # [-1, 125]
    Wcur = np.zeros((64, 64, d))
    Wprev = np.zeros((64, 64, d))
    mc = (lag_cur >= -2) & (lag_cur <= LMAX)
    mp = (lag_prev >= -2) & (lag_prev <= LMAX)
    Wcur[mc] = gp[(lag_cur + 2)[mc]]
    Wprev[mp] = gp[(lag_prev + 2)[mp]]
    return Wcur, Wprev


def _make_in_maps(v, l_filter, r_filter, n_cores=N_CORES):
    import ml_dtypes
    f8 = ml_dtypes.float8_e4m3
    Wcur, Wprev = _build_filters(l_filter, r_filter)
    # w lhsT layout: [part (half, j), pairall, col (cur|prev, i)]
    wsc = (Wcur * WSCALE).astype(np.float32)       # (64j, 64i, D)
    wsp = (Wprev * WSCALE).astype(np.float32)
    wall = np.empty((2, 64, D // 2, 2, 64), np.float32)
    # ch = 2*qall + half ; wall[half, j, qall, 0, i] = Wcur[j, i, ch]
    wall[0, :, :, 0, :] = wsc[:, :, 0::2].transpose(0, 2, 1)
    wall[0, :, :, 1, :] = wsp[:, :, 0::2].transpose(0, 2, 1)
    wall[1, :, :, 0, :] = wsc[:, :, 1::2].transpose(0, 2, 1)
    wall[1, :, :, 1, :] = wsp[:, :, 1::2].transpose(0, 2, 1)
    wall8 = wall.astype(f8)

    vr = np.asarray(v, dtype=np.float32).reshape(BH, T, D)
    vq = vr.astype(f8)                             # RNE quantize once
    # windows: frames 2..1025 (frames 1024,1025 zero) -> (BH, 16, 64, D)
    vpad = np.zeros((BH, NWIN * 64, D), f8)
    vpad[:, :T - 2] = vq[:, 2:, :]
    vw = vpad.reshape(BH, NWIN, 64, D)
    # xall[half, j, qall, win, bh] = vw[bh, win, j, 2*qall+half]
    xall = vw.transpose(3, 2, 1, 0).reshape(D // 2, 2, 64, NWIN, BH)
    xall = np.ascontiguousarray(xall.transpose(1, 2, 0, 3, 4))

    in_maps = []
    qc = PAIRS
    for cid in range(n_cores):
        sl = slice(cid * qc, (cid + 1) * qc)
        in_maps.append({
            "x": np.ascontiguousarray(xall[:, :, sl]).reshape(128, qc * FREE),
            "w": np.ascontiguousarray(wall8[:, :, sl]).reshape(128, qc * WCOL),
        })
    return in_maps


def kernel(v: np.ndarray, l_filter: np.ndarray, r_filter: np.ndarray,
           **_unused) -> np.ndarray:
    nc = _get_nc(PAIRS)
    in_maps = _make_in_maps(v, l_filter, r_filter)
    res = bass_utils.run_bass_kernel_spmd(nc, in_maps,
                                          core_ids=list(range(N_CORES)))
    vr = np.asarray(v, dtype=np.float32).reshape(BH, T, D)
    c = np.empty((BH, T, D), np.float32)
    qc = PAIRS
    for cid in range(N_CORES):
        yc = np.asarray(res.results[cid]["y"]).astype(np.float32)
        # [part (half, i), pair, win, bh] -> (bh, win, i, pair, half)
        yc = yc.reshape(2, 64, qc, NWIN, BH).transpose(4, 3, 1, 2, 0)
        c[:, :, 2 * qc * cid:2 * qc * (cid + 1)] = (
            yc.reshape(BH, T, 2 * qc) * (1.0 / WSCALE))

    # block-0 boundary: the dropped window -1 holds v[0], v[1] at rows 62, 63
    _, Wprev = _build_filters(l_filter, r_filter)
    corr0 = (np.einsum("id,nd->nid", Wprev[62], vr[:, 0, :].astype(np.float64))
             + np.einsum("id,nd->nid", Wprev[63], vr[:, 1, :].astype(np.float64)))
    c[:, 0:64, :] += corr0.astype(np.float32)
    out = vr + c
    return out.reshape(B, H, T, D)


# revision 7
# speedup vs baseline: 1.4521x; 1.0410x over previous
"""DFSMN (order-9 IIR + 2-tap lookahead FIR along frames) on 8 Trainium2 cores.

Math: per (b, h, d) sequence the reference is an LTI filter along frames,
out = g_d * v (combined impulse response g, lags -2..inf, |g| below noise
past lag ~61).  The output is dominated by the identity tap (g[0] ~= 1,
all other taps ~0.05), so the kernel computes only the CORRECTION

    c = (G - I) v        (RMS(c) ~= 0.17 * RMS(out))

and the host adds back the exact fp32 v it already holds.  Every fp8
quantization error (x, w, y) then scales with RMS(c), not RMS(out):
end-to-end rel err ~= 8e-3 vs the 2e-2 gate, at HALF the bf16 traffic.

Blocking: 64-frame output blocks; block b needs lags -2..61 ->
windows b and b-1 of the (+2-shifted) input:

    c_blk[b] = Wcur_d^T @ win[b] + Wprev_d^T @ win[b-1]

with Wcur/Wprev [64x64] Toeplitz slices of g' = g - delta, scaled by 16
so the decaying tail clears the fp8 subnormal floor (host divides by 16).

Packing: 2 channels per 128 partitions (chA rows 0:64, chB rows 64:128),
PE quadrant tiling at (0,0)/(64,64) -> both channels' matmuls run
concurrently on the 128x128 array; windows x 64 (b,h) seqs ride the
free dim (16*64 = 1024 cols per channel-pair).

Per-core traffic (64 channels = 32 pairs): x 4.19MB + w 0.52MB +
y 4.19MB = 8.9MB fp8 -> ~25us HBM floor at 358 GB/s.

Window -1 of block 0 holds v[0:2] (rank-2): applied on the host in f64
together with nothing else -- the (G-I) form needs no other boundary fix.

Per-core tensors:
    x  [128, 32*1024] f8e4   col = q*1024 + win*64 + bh; part j<64 chA
                             frame j of win, j>=64 chB (ch = 2q, 2q+1)
    w  [128, 32*128]  f8e4   col = q*128 + {0:64 cur | 64:128 prev} lhsT
    y  [128, 32*1024] f8e4   col = q*1024 + blk*64 + bh; part i<64 chA
                             frame 64*blk+i, i>=64 chB; value = 16*c
"""

import numpy as np

import concourse.bass as bass
import concourse.bacc as bacc
import concourse.mybir as mybir
from concourse import tile
from concourse import bass_utils

B, H, T, D = 16, 4, 1024, 512
N_CORES = 8
DC = D // N_CORES          # 64 channels per core
PAIRS = DC // 2            # 32 channel pairs per core
BH = B * H                 # 64 sequences (matmul free dim)
NWIN = 16                  # 64-frame windows
FREE = NWIN * BH           # 1024 matmul free dim per pair
WCOL = 128                 # w cols per pair: cur [64] + prev [64]
LMAX = 61                  # last kept lag of the impulse response
WSCALE = 16.0              # host-side weight scale (psum = 16*c)
F32 = mybir.dt.float32
BF16 = mybir.dt.bfloat16
F8 = mybir.dt.float8e4

_NC_CACHE: dict = {}


def _build_nc(pairs: int = PAIRS):
    nc = bacc.Bacc("TRN2", target_bir_lowering=False, debug=False)
    x = nc.dram_tensor("x", [128, pairs * FREE], F8, kind="ExternalInput")
    w = nc.dram_tensor("w", [128, pairs * WCOL], F8, kind="ExternalInput")
    y = nc.dram_tensor("y", [128, pairs * FREE], F8, kind="ExternalOutput")
    xap, wap, yap = x.ap(), w.ap(), y.ap()
    YG = 4                             # pairs per y-store (512KB)
    # ramped x load groups (pairs): small first tiles arrive early so the
    # matmul stream starts right after the PE warm-up burst
    GROUPS = [(0, 2), (2, 2), (4, 4), (8, 8), (16, 8), (24, 8)]
    gmap = {}                          # q -> (group_start, group_size)
    for s, n in GROUPS:
        for q in range(s, s + n):
            gmap[q] = (s, n)
    WGROUPS = [(0, 2), (2, 6), (8, 24)]

    with tile.TileContext(nc) as tc:
        with tc.tile_pool(name="xp", bufs=6) as xp, \
             tc.tile_pool(name="wp", bufs=len(WGROUPS)) as wp, \
             tc.tile_pool(name="op", bufs=3) as op, \
             tc.tile_pool(name="dp", bufs=1) as dp, \
             tc.tile_pool(name="pp", bufs=3, space="PSUM") as pp, \
             tc.tile_pool(name="pwp", bufs=1, space="PSUM") as pwp:
            # PE warm-up: HAM clock gate keeps PE at 1.2 GHz until ~3.4us of
            # sustained activity; burn the DMA-head time on dummy matmuls.
            dummy = dp.tile([128, 512], BF16, name="dummy")
            nc.gpsimd.memset(dummy, 0.0)
            pw = pwp.tile([128, 512], F32, name="pw")
            for _ in range(6):
                nc.tensor.matmul(pw, lhsT=dummy[:, 0:128], rhs=dummy,
                                 start=True, stop=True)
            # w (0.52MB) loads early on the scalar HWDGE queue, stays resident
            wtiles = {}
            for s, n in WGROUPS:
                wt = wp.tile([128, n * WCOL], F8, name="wt")
                wsrc = wap.copy()
                wsrc.ap = wsrc.ap[:0] + [[pairs * WCOL, 128], [1, n * WCOL]]
                wsrc.offset = s * WCOL
                nc.scalar.dma_start(out=wt, in_=wsrc)
                wtiles[s] = wt
            wmap = {}
            for s, n in WGROUPS:
                for q in range(s, s + n):
                    wmap[q] = (s, n)
            xt = yt = None
            xbase = None
            for q in range(pairs):
                if gmap[q][0] == q:
                    s, n = gmap[q]
                    xt = xp.tile([128, n * FREE], F8, name="xt")
                    src = xap.copy()
                    src.ap = src.ap[:0] + [[pairs * FREE, 128], [1, n * FREE]]
                    src.offset = s * FREE
                    nc.sync.dma_start(out=xt, in_=src)
                    xbase = s
                if q % YG == 0:
                    yt = op.tile([128, YG * FREE], F8, name="yt")
                xv = xt[:, (q - xbase) * FREE:(q - xbase + 1) * FREE]
                ws, _ = wmap[q]
                wt = wtiles[ws]
                wo = (q - ws) * WCOL
                ps = pp.tile([128, FREE], F32, name="ps")
                # quadrant-tiled matmuls: chA on (0,0), chB on (64,64) run
                # concurrently; cur covers all 16 blocks, prev covers blocks
                # 1..15 via the 64-col shift.  Block 0's prev-window term is
                # rank-2 in v[0:2] and applied on the host.  Cols 0:64 never
                # see a stop=True -> skip the sim's accumulation-group check.
                # Each matmul's PSUM output must fit one 2KB bank -> split
                # the 1024-col free dim into 512-col halves.  Issue order
                # strictly alternates the (0,0) / (64,64) PE quadrants so
                # every LDWEIGHTS targets a row group that differs from the
                # in-flight matmul's and gets pulled ahead (hidden).
                HB = FREE // 2
                for c0, c1, wd, st in (
                        (0, HB, 0, True),          # cur, half 0
                        (HB, FREE, 0, True),       # cur, half 1
                        (64, HB, 64, False),       # prev, half 0
                        (HB, FREE, 64, False)):    # prev, half 1
                    sh = 0 if st else 64
                    for base in (0, 64):
                        nc.tensor.matmul(
                            ps[base:base + 64, c0:c1],
                            lhsT=wt[base:base + 64, wo + wd:wo + wd + 64],
                            rhs=xv[base:base + 64, c0 - sh:c1 - sh],
                            start=st, stop=not st, skip_group_check=True)
                # alternate PSUM evacuation between DVE and ACT
                ysl = yt[:, (q % YG) * FREE:(q % YG + 1) * FREE]
                if q % 2 == 0:
                    nc.vector.tensor_copy(ysl, ps)
                else:
                    nc.scalar.copy(ysl, ps)
                if q % YG == YG - 1:
                    g = q // YG
                    dst = yap.copy()
                    dst.ap = dst.ap[:0] + [[pairs * FREE, 128], [1, YG * FREE]]
                    dst.offset = g * YG * FREE
                    nc.gpsimd.dma_start(out=dst, in_=yt)
                    # filler matmul bridges DMA-jitter stalls so the HAM
                    # clock gate never re-throttles mid-stream
                    if q < pairs - 1:
                        nc.tensor.matmul(pw, lhsT=dummy[:, 0:128],
                                         rhs=dummy, start=True, stop=True)
    nc.compile()
    return nc


def _get_nc(pairs: int = PAIRS):
    if pairs not in _NC_CACHE:
        _NC_CACHE[pairs] = _build_nc(pairs)
    return _NC_CACHE[pairs]


def _build_filters(l_filter: np.ndarray, r_filter: np.ndarray):
    """Returns Wcur, Wprev [64, 64, D] float64 lhsT Toeplitz blocks of the
    correction filter g' = g - delta (unscaled), truncated past lag LMAX."""
    c = l_filter[1:].astype(np.float64)            # (9, D) IIR coeffs
    d = c.shape[1]
    n_a = LMAX + 3
    a = np.zeros((n_a, d))
    a[0] = 1.0
    for n in range(1, n_a):
        for k in range(1, min(9, n) + 1):
            a[n] += c[k - 1] * a[n - k]
    q0 = 1.0 + l_filter[0].astype(np.float64)
    q1 = r_filter[0].astype(np.float64)
    q2 = r_filter[1].astype(np.float64)

    # gp[i] = correction tap at lag i-2, i in [0, LMAX+2]
    gp = np.zeros((LMAX + 3, d))
    gp[0] = q2 * a[0]
    gp[1] = q1 * a[0] + q2 * a[1]
    for lag in range(0, LMAX + 1):
        gp[lag + 2] = q0 * a[lag] + q1 * a[lag + 1] + q2 * a[lag + 2]
    gp[2] -= 1.0                                   # subtract identity

    jj = np.arange(64)[:, None]
    ii = np.arange(64)[None, :]
    lag_cur = ii - jj - 2                          # [-65, 61]
    lag_prev = ii - jj + 62                        # [-1, 125]
    Wcur = np.zeros((64, 64, d))
    Wprev = np.zeros((64, 64, d))
    mc = (lag_cur >= -2) & (lag_cur <= LMAX)
    mp = (lag_prev >= -2) & (lag_prev <= LMAX)
    Wcur[mc] = gp[(lag_cur + 2)[mc]]
    Wprev[mp] = gp[(lag_prev + 2)[mp]]
    return Wcur, Wprev


def _make_in_maps(v, l_filter, r_filter, n_cores=N_CORES):
    import ml_dtypes
    f8 = ml_dtypes.float8_e4m3
    Wcur, Wprev = _build_filters(l_filter, r_filter)
    # w lhsT layout: [part (half, j), pairall, col (cur|prev, i)]
    wsc = (Wcur * WSCALE).astype(np.float32)       # (64j, 64i, D)
    wsp = (Wprev * WSCALE).astype(np.float32)
    wall = np.empty((2, 64, D // 2, 2, 64), np.float32)
    # ch = 2*qall + half ; wall[half, j, qall, 0, i] = Wcur[j, i, ch]
    wall[0, :, :, 0, :] = wsc[:, :, 0::2].transpose(0, 2, 1)
    wall[0, :, :, 1, :] = wsp[:, :, 0::2].transpose(0, 2, 1)
    wall[1, :, :, 0, :] = wsc[:, :, 1::2].transpose(0, 2, 1)
    wall[1, :, :, 1, :] = wsp[:, :, 1::2].transpose(0, 2, 1)
    wall8 = wall.astype(f8)

    vr = np.asarray(v, dtype=np.float32).reshape(BH, T, D)
    vq = vr.astype(f8)                             # RNE quantize once
    # windows: frames 2..1025 (frames 1024,1025 zero) -> (BH, 16, 64, D)
    vpad = np.zeros((BH, NWIN * 64, D), f8)
    vpad[:, :T - 2] = vq[:, 2:, :]
    vw = vpad.reshape(BH, NWIN, 64, D)
    # xall[half, j, qall, win, bh] = vw[bh, win, j, 2*qall+half]
    xall = vw.transpose(3, 2, 1, 0).reshape(D // 2, 2, 64, NWIN, BH)
    xall = np.ascontiguousarray(xall.transpose(1, 2, 0, 3, 4))

    in_maps = []
    qc = PAIRS
    for cid in range(n_cores):
        sl = slice(cid * qc, (cid + 1) * qc)
        in_maps.append({
            "x": np.ascontiguousarray(xall[:, :, sl]).reshape(128, qc * FREE),
            "w": np.ascontiguousarray(wall8[:, :, sl]).reshape(128, qc * WCOL),
        })
    return in_maps


def kernel(v: np.ndarray, l_filter: np.ndarray, r_filter: np.ndarray,
           **_unused) -> np.ndarray:
    nc = _get_nc(PAIRS)
    in_maps = _make_in_maps(v, l_filter, r_filter)
    res = bass_utils.run_bass_kernel_spmd(nc, in_maps,
                                          core_ids=list(range(N_CORES)))
    vr = np.asarray(v, dtype=np.float32).reshape(BH, T, D)
    c = np.empty((BH, T, D), np.float32)
    qc = PAIRS
    for cid in range(N_CORES):
        yc = np.asarray(res.results[cid]["y"]).astype(np.float32)
        # [part (half, i), pair, win, bh] -> (bh, win, i, pair, half)
        yc = yc.reshape(2, 64, qc, NWIN, BH).transpose(4, 3, 1, 2, 0)
        c[:, :, 2 * qc * cid:2 * qc * (cid + 1)] = (
            yc.reshape(BH, T, 2 * qc) * (1.0 / WSCALE))

    # block-0 boundary: the dropped window -1 holds v[0], v[1] at rows 62, 63
    _, Wprev = _build_filters(l_filter, r_filter)
    corr0 = (np.einsum("id,nd->nid", Wprev[62], vr[:, 0, :].astype(np.float64))
             + np.einsum("id,nd->nid", Wprev[63], vr[:, 1, :].astype(np.float64)))
    c[:, 0:64, :] += corr0.astype(np.float32)
    out = vr + c
    return out.reshape(B, H, T, D)


# revision 10
# speedup vs baseline: 1.4894x; 1.0257x over previous
"""DFSMN (order-9 IIR + 2-tap lookahead FIR along frames) on 8 Trainium2 cores.

Math: per (b, h, d) sequence the reference is an LTI filter along frames,
out = g_d * v (combined impulse response g, lags -2..inf, |g| below noise
past lag ~61).  The output is dominated by the identity tap (g[0] ~= 1,
all other taps ~0.05), so the kernel computes only the CORRECTION

    c = (G - I) v        (RMS(c) ~= 0.17 * RMS(out))

and the host adds back the exact fp32 v it already holds.  Every fp8
quantization error (x, w, y) then scales with RMS(c), not RMS(out):
end-to-end rel err ~= 8e-3 vs the 2e-2 gate, at HALF the bf16 traffic.

Blocking: 64-frame output blocks; block b needs lags -2..61 ->
windows b and b-1 of the (+2-shifted) input:

    c_blk[b] = Wcur_d^T @ win[b] + Wprev_d^T @ win[b-1]

with Wcur/Wprev [64x64] Toeplitz slices of g' = g - delta, scaled by 16
so the decaying tail clears the fp8 subnormal floor (host divides by 16).

Packing: 2 channels per 128 partitions (chA rows 0:64, chB rows 64:128),
PE quadrant tiling at (0,0)/(64,64) -> both channels' matmuls run
concurrently on the 128x128 array; windows x 64 (b,h) seqs ride the
free dim (16*64 = 1024 cols per channel-pair).

Per-core traffic (64 channels = 32 pairs): x 4.19MB + w 0.52MB +
y 4.19MB = 8.9MB fp8 -> ~25us HBM floor at 358 GB/s.

Window -1 of block 0 holds v[0:2] (rank-2): applied on the host in f64
together with nothing else -- the (G-I) form needs no other boundary fix.

Per-core tensors:
    x  [128, 32*1024] f8e4   col = q*1024 + win*64 + bh; part j<64 chA
                             frame j of win, j>=64 chB (ch = 2q, 2q+1)
    w  [128, 32*128]  f8e4   col = q*128 + {0:64 cur | 64:128 prev} lhsT
    y  [128, 32*1024] f8e4   col = q*1024 + blk*64 + bh; part i<64 chA
                             frame 64*blk+i, i>=64 chB; value = 16*c
"""

import numpy as np

import concourse.bass as bass
import concourse.bacc as bacc
import concourse.mybir as mybir
from concourse import tile
from concourse import bass_utils

B, H, T, D = 16, 4, 1024, 512
N_CORES = 8
DC = D // N_CORES          # 64 channels per core
PAIRS = DC // 2            # 32 channel pairs per core
BH = B * H                 # 64 sequences (matmul free dim)
NWIN = 16                  # 64-frame windows
FREE = NWIN * BH           # 1024 matmul free dim per pair
WCOL = 128                 # w cols per pair: cur [64] + prev [64]
LMAX = 61                  # last kept lag of the impulse response
WSCALE = 16.0              # host-side weight scale (psum = 16*c)
F32 = mybir.dt.float32
BF16 = mybir.dt.bfloat16
F8 = mybir.dt.float8e4

_NC_CACHE: dict = {}


def _build_nc(pairs: int = PAIRS):
    nc = bacc.Bacc("TRN2", target_bir_lowering=False, debug=False)
    x = nc.dram_tensor("x", [128, pairs * FREE], F8, kind="ExternalInput")
    w = nc.dram_tensor("w", [128, pairs * WCOL], F8, kind="ExternalInput")
    y = nc.dram_tensor("y", [128, pairs * FREE], F8, kind="ExternalOutput")
    xap, wap, yap = x.ap(), w.ap(), y.ap()
    # y-store groups (pairs): big in the middle, small last so the final
    # store's transfer+receipt tail is short
    YGROUPS = [(0, 8), (8, 8), (16, 8), (24, 6), (30, 2)]
    ymap = {}
    for s, n in YGROUPS:
        ymap[s + n - 1] = (s, n)       # trigger store on the group's last pair
    ystart = {s: (s, n) for s, n in YGROUPS}
    # ramped x load groups (pairs): small first tiles arrive early so the
    # matmul stream starts right after the PE warm-up burst
    GROUPS = [(0, 2), (2, 2), (4, 4), (8, 12), (20, 12)]
    gmap = {}                          # q -> (group_start, group_size)
    for s, n in GROUPS:
        for q in range(s, s + n):
            gmap[q] = (s, n)
    WGROUPS = [(0, 4), (4, 28)]

    with tile.TileContext(nc) as tc:
        with tc.tile_pool(name="xp", bufs=6) as xp, \
             tc.tile_pool(name="wp", bufs=len(WGROUPS)) as wp, \
             tc.tile_pool(name="op", bufs=3) as op, \
             tc.tile_pool(name="dp", bufs=1) as dp, \
             tc.tile_pool(name="pp", bufs=3, space="PSUM") as pp, \
             tc.tile_pool(name="pwp", bufs=1, space="PSUM") as pwp:
            # PE warm-up: HAM clock gate keeps PE at 1.2 GHz until ~3.4us of
            # sustained activity; burn the DMA-head time on dummy matmuls.
            dummy = dp.tile([128, 512], BF16, name="dummy")
            nc.gpsimd.memset(dummy, 0.0)
            pw = pwp.tile([128, 512], F32, name="pw")
            for _ in range(6):
                nc.tensor.matmul(pw, lhsT=dummy[:, 0:128], rhs=dummy,
                                 start=True, stop=True)
            # w (0.52MB) loads early on the scalar HWDGE queue, stays resident
            wtiles = {}
            for s, n in WGROUPS:
                wt = wp.tile([128, n * WCOL], F8, name="wt")
                wsrc = wap.copy()
                wsrc.ap = wsrc.ap[:0] + [[pairs * WCOL, 128], [1, n * WCOL]]
                wsrc.offset = s * WCOL
                nc.scalar.dma_start(out=wt, in_=wsrc)
                wtiles[s] = wt
            wmap = {}
            for s, n in WGROUPS:
                for q in range(s, s + n):
                    wmap[q] = (s, n)
            xt = yt = None
            xbase = None
            for q in range(pairs):
                if gmap[q][0] == q:
                    s, n = gmap[q]
                    xt = xp.tile([128, n * FREE], F8, name="xt")
                    src = xap.copy()
                    src.ap = src.ap[:0] + [[pairs * FREE, 128], [1, n * FREE]]
                    src.offset = s * FREE
                    nc.sync.dma_start(out=xt, in_=src)
                    xbase = s
                if q in ystart:
                    ys, yn = ystart[q]
                    yt = op.tile([128, yn * FREE], F8, name="yt")
                xv = xt[:, (q - xbase) * FREE:(q - xbase + 1) * FREE]
                ws, _ = wmap[q]
                wt = wtiles[ws]
                wo = (q - ws) * WCOL
                ps = pp.tile([128, FREE], F32, name="ps")
                # quadrant-tiled matmuls: chA on (0,0), chB on (64,64) run
                # concurrently; cur covers all 16 blocks, prev covers blocks
                # 1..15 via the 64-col shift.  Block 0's prev-window term is
                # rank-2 in v[0:2] and applied on the host.  Cols 0:64 never
                # see a stop=True -> skip the sim's accumulation-group check.
                # Each matmul's PSUM output must fit one 2KB bank -> split
                # the 1024-col free dim into 512-col halves.  Issue order
                # strictly alternates the (0,0) / (64,64) PE quadrants so
                # every LDWEIGHTS targets a row group that differs from the
                # in-flight matmul's and gets pulled ahead (hidden).
                HB = FREE // 2
                for c0, c1, wd, st in (
                        (0, HB, 0, True),          # cur, half 0
                        (HB, FREE, 0, True),       # cur, half 1
                        (64, HB, 64, False),       # prev, half 0
                        (HB, FREE, 64, False)):    # prev, half 1
                    sh = 0 if st else 64
                    for base in (0, 64):
                        nc.tensor.matmul(
                            ps[base:base + 64, c0:c1],
                            lhsT=wt[base:base + 64, wo + wd:wo + wd + 64],
                            rhs=xv[base:base + 64, c0 - sh:c1 - sh],
                            start=st, stop=not st, skip_group_check=True)
                # alternate PSUM evacuation between DVE and ACT
                ysl = yt[:, (q - ys) * FREE:(q - ys + 1) * FREE]
                if q % 2 == 0:
                    nc.vector.tensor_copy(ysl, ps)
                else:
                    nc.scalar.copy(ysl, ps)
                if q in ymap:
                    gs, gn = ymap[q]
                    dst = yap.copy()
                    dst.ap = dst.ap[:0] + [[pairs * FREE, 128], [1, gn * FREE]]
                    dst.offset = gs * FREE
                    nc.gpsimd.dma_start(out=dst, in_=yt)
                    # filler matmul bridges DMA-jitter stalls so the HAM
                    # clock gate never re-throttles mid-stream
                    if q < pairs - 1:
                        nc.tensor.matmul(pw, lhsT=dummy[:, 0:128],
                                         rhs=dummy, start=True, stop=True)
    nc.compile()
    return nc


def _get_nc(pairs: int = PAIRS):
    if pairs not in _NC_CACHE:
        _NC_CACHE[pairs] = _build_nc(pairs)
    return _NC_CACHE[pairs]


def _build_filters(l_filter: np.ndarray, r_filter: np.ndarray):
    """Returns Wcur, Wprev [64, 64, D] float64 lhsT Toeplitz blocks of the
    correction filter g' = g - delta (unscaled), truncated past lag LMAX."""
    c = l_filter[1:].astype(np.float64)            # (9, D) IIR coeffs
    d = c.shape[1]
    n_a = LMAX + 3
    a = np.zeros((n_a, d))
    a[0] = 1.0
    for n in range(1, n_a):
        for k in range(1, min(9, n) + 1):
            a[n] += c[k - 1] * a[n - k]
    q0 = 1.0 + l_filter[0].astype(np.float64)
    q1 = r_filter[0].astype(np.float64)
    q2 = r_filter[1].astype(np.float64)

    # gp[i] = correction tap at lag i-2, i in [0, LMAX+2]
    gp = np.zeros((LMAX + 3, d))
    gp[0] = q2 * a[0]
    gp[1] = q1 * a[0] + q2 * a[1]
    for lag in range(0, LMAX + 1):
        gp[lag + 2] = q0 * a[lag] + q1 * a[lag + 1] + q2 * a[lag + 2]
    gp[2] -= 1.0                                   # subtract identity

    jj = np.arange(64)[:, None]
    ii = np.arange(64)[None, :]
    lag_cur = ii - jj - 2                          # [-65, 61]
    lag_prev = ii - jj + 62                        # [-1, 125]
    Wcur = np.zeros((64, 64, d))
    Wprev = np.zeros((64, 64, d))
    mc = (lag_cur >= -2) & (lag_cur <= LMAX)
    mp = (lag_prev >= -2) & (lag_prev <= LMAX)
    Wcur[mc] = gp[(lag_cur + 2)[mc]]
    Wprev[mp] = gp[(lag_prev + 2)[mp]]
    return Wcur, Wprev


def _make_in_maps(v, l_filter, r_filter, n_cores=N_CORES):
    import ml_dtypes
    f8 = ml_dtypes.float8_e4m3
    Wcur, Wprev = _build_filters(l_filter, r_filter)
    # w lhsT layout: [part (half, j), pairall, col (cur|prev, i)]
    wsc = (Wcur * WSCALE).astype(np.float32)       # (64j, 64i, D)
    wsp = (Wprev * WSCALE).astype(np.float32)
    wall = np.empty((2, 64, D // 2, 2, 64), np.float32)
    # ch = 2*qall + half ; wall[half, j, qall, 0, i] = Wcur[j, i, ch]
    wall[0, :, :, 0, :] = wsc[:, :, 0::2].transpose(0, 2, 1)
    wall[0, :, :, 1, :] = wsp[:, :, 0::2].transpose(0, 2, 1)
    wall[1, :, :, 0, :] = wsc[:, :, 1::2].transpose(0, 2, 1)
    wall[1, :, :, 1, :] = wsp[:, :, 1::2].transpose(0, 2, 1)
    wall8 = wall.astype(f8)

    vr = np.asarray(v, dtype=np.float32).reshape(BH, T, D)
    vq = vr.astype(f8)                             # RNE quantize once
    # windows: frames 2..1025 (frames 1024,1025 zero) -> (BH, 16, 64, D)
    vpad = np.zeros((BH, NWIN * 64, D), f8)
    vpad[:, :T - 2] = vq[:, 2:, :]
    vw = vpad.reshape(BH, NWIN, 64, D)
    # xall[half, j, qall, win, bh] = vw[bh, win, j, 2*qall+half]
    xall = vw.transpose(3, 2, 1, 0).reshape(D // 2, 2, 64, NWIN, BH)
    xall = np.ascontiguousarray(xall.transpose(1, 2, 0, 3, 4))

    in_maps = []
    qc = PAIRS
    for cid in range(n_cores):
        sl = slice(cid * qc, (cid + 1) * qc)
        in_maps.append({
            "x": np.ascontiguousarray(xall[:, :, sl]).reshape(128, qc * FREE),
            "w": np.ascontiguousarray(wall8[:, :, sl]).reshape(128, qc * WCOL),
        })
    return in_maps


def kernel(v: np.ndarray, l_filter: np.ndarray, r_filter: np.ndarray,
           **_unused) -> np.ndarray:
    nc = _get_nc(PAIRS)
    in_maps = _make_in_maps(v, l_filter, r_filter)
    res = bass_utils.run_bass_kernel_spmd(nc, in_maps,
                                          core_ids=list(range(N_CORES)))
    vr = np.asarray(v, dtype=np.float32).reshape(BH, T, D)
    c = np.empty((BH, T, D), np.float32)
    qc = PAIRS
    for cid in range(N_CORES):
        yc = np.asarray(res.results[cid]["y"]).astype(np.float32)
        # [part (half, i), pair, win, bh] -> (bh, win, i, pair, half)
        yc = yc.reshape(2, 64, qc, NWIN, BH).transpose(4, 3, 1, 2, 0)
        c[:, :, 2 * qc * cid:2 * qc * (cid + 1)] = (
            yc.reshape(BH, T, 2 * qc) * (1.0 / WSCALE))

    # block-0 boundary: the dropped window -1 holds v[0], v[1] at rows 62, 63
    _, Wprev = _build_filters(l_filter, r_filter)
    corr0 = (np.einsum("id,nd->nid", Wprev[62], vr[:, 0, :].astype(np.float64))
             + np.einsum("id,nd->nid", Wprev[63], vr[:, 1, :].astype(np.float64)))
    c[:, 0:64, :] += corr0.astype(np.float32)
    out = vr + c
    return out.reshape(B, H, T, D)


# revision 14
# speedup vs baseline: 1.5376x; 1.0324x over previous
"""DFSMN (order-9 IIR + 2-tap lookahead FIR along frames) on 8 Trainium2 cores.

Math: per (b, h, d) sequence the reference is an LTI filter along frames,
out = g_d * v (combined impulse response g, lags -2..inf, |g| below noise
past lag ~61).  The output is dominated by the identity tap (g[0] ~= 1,
all other taps ~0.05), so the kernel computes only the CORRECTION

    c = (G - I) v        (RMS(c) ~= 0.17 * RMS(out))

and the host adds back the exact fp32 v it already holds.  Every fp8
quantization error (x, w, y) then scales with RMS(c), not RMS(out):
end-to-end rel err ~= 8e-3 vs the 2e-2 gate, at HALF the bf16 traffic.

Blocking: 64-frame output blocks; block b needs lags -2..61 ->
windows b and b-1 of the (+2-shifted) input:

    c_blk[b] = Wcur_d^T @ win[b] + Wprev_d^T @ win[b-1]

with Wcur/Wprev [64x64] Toeplitz slices of g' = g - delta, scaled by 16
so the decaying tail clears the fp8 subnormal floor (host divides by 16).

Packing: 2 channels per 128 partitions (chA rows 0:64, chB rows 64:128),
PE quadrant tiling at (0,0)/(64,64) -> both channels' matmuls run
concurrently on the 128x128 array; windows x 64 (b,h) seqs ride the
free dim (16*64 = 1024 cols per channel-pair).

Per-core traffic (64 channels = 32 pairs): x 4.19MB + w 0.52MB +
y 4.19MB = 8.9MB fp8 -> ~25us HBM floor at 358 GB/s.

Window -1 of block 0 holds v[0:2] (rank-2): applied on the host in f64
together with nothing else -- the (G-I) form needs no other boundary fix.

Per-core tensors:
    x  [128, 32*1024] f8e4   col = q*1024 + win*64 + bh; part j<64 chA
                             frame j of win, j>=64 chB (ch = 2q, 2q+1)
    w  [128, 32*128]  f8e4   col = q*128 + {0:64 cur | 64:128 prev} lhsT
    y  [128, 32*1024] f8e4   col = q*1024 + blk*64 + bh; part i<64 chA
                             frame 64*blk+i, i>=64 chB; value = 16*c
"""

import numpy as np

import concourse.bass as bass
import concourse.bacc as bacc
import concourse.mybir as mybir
from concourse import tile
from concourse import bass_utils

B, H, T, D = 16, 4, 1024, 512
N_CORES = 8
DC = D // N_CORES          # 64 channels per core
PAIRS = DC // 2            # 32 channel pairs per core
BH = B * H                 # 64 sequences (matmul free dim)
NWIN = 16                  # 64-frame windows
FREE = NWIN * BH           # 1024 matmul free dim per pair
WCOL = 128                 # w cols per pair: cur [64] + prev [64]
LMAX = 61                  # last kept lag of the impulse response
WSCALE = 16.0              # host-side weight scale (psum = 16*c)
F32 = mybir.dt.float32
BF16 = mybir.dt.bfloat16
F8 = mybir.dt.float8e4

_NC_CACHE: dict = {}


def _build_nc(pairs: int = PAIRS):
    nc = bacc.Bacc("TRN2", target_bir_lowering=False, debug=False)
    x = nc.dram_tensor("x", [128, pairs * FREE], F8, kind="ExternalInput")
    w = nc.dram_tensor("w", [128, pairs * WCOL], F8, kind="ExternalInput")
    y = nc.dram_tensor("y", [128, pairs * FREE], F8, kind="ExternalOutput")
    xap, wap, yap = x.ap(), w.ap(), y.ap()
    # y-store groups (pairs): big in the middle, small last so the final
    # store's transfer+receipt tail is short
    YGROUPS = [(0, 8), (8, 8), (16, 8), (24, 6), (30, 2)]
    ymap = {}
    for s, n in YGROUPS:
        ymap[s + n - 1] = (s, n)       # trigger store on the group's last pair
    ystart = {s: (s, n) for s, n in YGROUPS}
    # ramped x load groups (pairs): small first tiles arrive early so the
    # matmul stream starts right after the PE warm-up burst
    GROUPS = [(0, 1), (1, 1), (2, 2), (4, 8), (12, 10), (22, 10)]
    gmap = {}                          # q -> (group_start, group_size)
    for s, n in GROUPS:
        for q in range(s, s + n):
            gmap[q] = (s, n)
    WGROUPS = [(0, 4), (4, 28)]

    with tile.TileContext(nc) as tc:
        with tc.tile_pool(name="xp", bufs=6) as xp, \
             tc.tile_pool(name="wp", bufs=len(WGROUPS)) as wp, \
             tc.tile_pool(name="op", bufs=3) as op, \
             tc.tile_pool(name="dp", bufs=1) as dp, \
             tc.tile_pool(name="pp", bufs=3, space="PSUM") as pp, \
             tc.tile_pool(name="pwp", bufs=1, space="PSUM") as pwp:
            # PE warm-up: HAM clock gate keeps PE at 1.2 GHz until ~3.4us of
            # sustained activity; burn the DMA-head time on dummy matmuls.
            # memset on DVE -- its preamble finishes ~3us before GpSimd's,
            # so the warm-up burst starts right after the framework barrier
            # instead of idling behind the first x DMA.
            dummy = dp.tile([128, 512], BF16, name="dummy")
            nc.vector.memset(dummy, 0.0)
            pw = pwp.tile([128, 512], F32, name="pw")
            for _ in range(3):
                nc.tensor.matmul(pw, lhsT=dummy[:, 0:128], rhs=dummy,
                                 start=True, stop=True)
            # w (0.52MB) loads early on the scalar HWDGE queue, stays resident
            wtiles = {}
            for s, n in WGROUPS:
                wt = wp.tile([128, n * WCOL], F8, name="wt")
                wsrc = wap.copy()
                wsrc.ap = wsrc.ap[:0] + [[pairs * WCOL, 128], [1, n * WCOL]]
                wsrc.offset = s * WCOL
                nc.scalar.dma_start(out=wt, in_=wsrc)
                wtiles[s] = wt
            wmap = {}
            for s, n in WGROUPS:
                for q in range(s, s + n):
                    wmap[q] = (s, n)
            xt = yt = None
            xbase = None
            for q in range(pairs):
                if gmap[q][0] == q:
                    s, n = gmap[q]
                    xt = xp.tile([128, n * FREE], F8, name="xt")
                    src = xap.copy()
                    src.ap = src.ap[:0] + [[pairs * FREE, 128], [1, n * FREE]]
                    src.offset = s * FREE
                    nc.sync.dma_start(out=xt, in_=src)
                    xbase = s
                if q in ystart:
                    ys, yn = ystart[q]
                    yt = op.tile([128, yn * FREE], F8, name="yt")
                xv = xt[:, (q - xbase) * FREE:(q - xbase + 1) * FREE]
                ws, _ = wmap[q]
                wt = wtiles[ws]
                wo = (q - ws) * WCOL
                ps = pp.tile([128, FREE], F32, name="ps")
                # quadrant-tiled matmuls: chA on (0,0), chB on (64,64) run
                # concurrently; cur covers all 16 blocks, prev covers blocks
                # 1..15 via the 64-col shift.  Block 0's prev-window term is
                # rank-2 in v[0:2] and applied on the host.  Cols 0:64 never
                # see a stop=True -> skip the sim's accumulation-group check.
                # Each matmul's PSUM output must fit one 2KB bank -> split
                # the 1024-col free dim into 512-col halves.  Issue order
                # strictly alternates the (0,0) / (64,64) PE quadrants so
                # every LDWEIGHTS targets a row group that differs from the
                # in-flight matmul's and gets pulled ahead (hidden).
                HB = FREE // 2
                # odd pairs swap their PSUM row halves (PE tiles (0,64) and
                # (64,0) instead of (0,0)/(64,64)) so consecutive pairs touch
                # disjoint array quadrants; the host unswaps when unpacking.
                swap = 64 if (q % 2) else 0
                for c0, c1, wd, st in (
                        (0, HB, 0, True),          # cur, half 0
                        (HB, FREE, 0, True),       # cur, half 1
                        (64, HB, 64, False),       # prev, half 0
                        (HB, FREE, 64, False)):    # prev, half 1
                    sh = 0 if st else 64
                    for base in (0, 64):
                        ob = base ^ swap
                        nc.tensor.matmul(
                            ps[ob:ob + 64, c0:c1],
                            lhsT=wt[base:base + 64, wo + wd:wo + wd + 64],
                            rhs=xv[base:base + 64, c0 - sh:c1 - sh],
                            start=st, stop=not st, skip_group_check=True)
                # alternate PSUM evacuation between DVE and ACT
                ysl = yt[:, (q - ys) * FREE:(q - ys + 1) * FREE]
                if q % 2 == 0:
                    nc.vector.tensor_copy(ysl, ps)
                else:
                    nc.scalar.copy(ysl, ps)
                if q in ymap:
                    gs, gn = ymap[q]
                    dst = yap.copy()
                    dst.ap = dst.ap[:0] + [[pairs * FREE, 128], [1, gn * FREE]]
                    dst.offset = gs * FREE
                    nc.gpsimd.dma_start(out=dst, in_=yt)
                    # filler matmul bridges DMA-jitter stalls so the HAM
                    # clock gate never re-throttles mid-stream
                    if q < pairs - 1:
                        nc.tensor.matmul(pw, lhsT=dummy[:, 0:128],
                                         rhs=dummy, start=True, stop=True)
    nc.compile()
    return nc


def _get_nc(pairs: int = PAIRS):
    if pairs not in _NC_CACHE:
        _NC_CACHE[pairs] = _build_nc(pairs)
    return _NC_CACHE[pairs]


def _build_filters(l_filter: np.ndarray, r_filter: np.ndarray):
    """Returns Wcur, Wprev [64, 64, D] float64 lhsT Toeplitz blocks of the
    correction filter g' = g - delta (unscaled), truncated past lag LMAX."""
    c = l_filter[1:].astype(np.float64)            # (9, D) IIR coeffs
    d = c.shape[1]
    n_a = LMAX + 3
    a = np.zeros((n_a, d))
    a[0] = 1.0
    for n in range(1, n_a):
        for k in range(1, min(9, n) + 1):
            a[n] += c[k - 1] * a[n - k]
    q0 = 1.0 + l_filter[0].astype(np.float64)
    q1 = r_filter[0].astype(np.float64)
    q2 = r_filter[1].astype(np.float64)

    # gp[i] = correction tap at lag i-2, i in [0, LMAX+2]
    gp = np.zeros((LMAX + 3, d))
    gp[0] = q2 * a[0]
    gp[1] = q1 * a[0] + q2 * a[1]
    for lag in range(0, LMAX + 1):
        gp[lag + 2] = q0 * a[lag] + q1 * a[lag + 1] + q2 * a[lag + 2]
    gp[2] -= 1.0                                   # subtract identity

    jj = np.arange(64)[:, None]
    ii = np.arange(64)[None, :]
    lag_cur = ii - jj - 2                          # [-65, 61]
    lag_prev = ii - jj + 62                        # [-1, 125]
    Wcur = np.zeros((64, 64, d))
    Wprev = np.zeros((64, 64, d))
    mc = (lag_cur >= -2) & (lag_cur <= LMAX)
    mp = (lag_prev >= -2) & (lag_prev <= LMAX)
    Wcur[mc] = gp[(lag_cur + 2)[mc]]
    Wprev[mp] = gp[(lag_prev + 2)[mp]]
    return Wcur, Wprev


def _make_in_maps(v, l_filter, r_filter, n_cores=N_CORES):
    import ml_dtypes
    f8 = ml_dtypes.float8_e4m3
    Wcur, Wprev = _build_filters(l_filter, r_filter)
    # w lhsT layout: [part (half, j), pairall, col (cur|prev, i)]
    wsc = (Wcur * WSCALE).astype(np.float32)       # (64j, 64i, D)
    wsp = (Wprev * WSCALE).astype(np.float32)
    wall = np.empty((2, 64, D // 2, 2, 64), np.float32)
    # ch = 2*qall + half ; wall[half, j, qall, 0, i] = Wcur[j, i, ch]
    wall[0, :, :, 0, :] = wsc[:, :, 0::2].transpose(0, 2, 1)
    wall[0, :, :, 1, :] = wsp[:, :, 0::2].transpose(0, 2, 1)
    wall[1, :, :, 0, :] = wsc[:, :, 1::2].transpose(0, 2, 1)
    wall[1, :, :, 1, :] = wsp[:, :, 1::2].transpose(0, 2, 1)
    wall8 = wall.astype(f8)

    vr = np.asarray(v, dtype=np.float32).reshape(BH, T, D)
    vq = vr.astype(f8)                             # RNE quantize once
    # windows: frames 2..1025 (frames 1024,1025 zero) -> (BH, 16, 64, D)
    vpad = np.zeros((BH, NWIN * 64, D), f8)
    vpad[:, :T - 2] = vq[:, 2:, :]
    vw = vpad.reshape(BH, NWIN, 64, D)
    # xall[half, j, qall, win, bh] = vw[bh, win, j, 2*qall+half]
    xall = vw.transpose(3, 2, 1, 0).reshape(D // 2, 2, 64, NWIN, BH)
    xall = np.ascontiguousarray(xall.transpose(1, 2, 0, 3, 4))

    in_maps = []
    qc = PAIRS
    for cid in range(n_cores):
        sl = slice(cid * qc, (cid + 1) * qc)
        in_maps.append({
            "x": np.ascontiguousarray(xall[:, :, sl]).reshape(128, qc * FREE),
            "w": np.ascontiguousarray(wall8[:, :, sl]).reshape(128, qc * WCOL),
        })
    return in_maps


def kernel(v: np.ndarray, l_filter: np.ndarray, r_filter: np.ndarray,
           **_unused) -> np.ndarray:
    nc = _get_nc(PAIRS)
    in_maps = _make_in_maps(v, l_filter, r_filter)
    res = bass_utils.run_bass_kernel_spmd(nc, in_maps,
                                          core_ids=list(range(N_CORES)))
    vr = np.asarray(v, dtype=np.float32).reshape(BH, T, D)
    c = np.empty((BH, T, D), np.float32)
    qc = PAIRS
    for cid in range(N_CORES):
        yc = np.asarray(res.results[cid]["y"]).astype(np.float32)
        # [part (half, i), pair, win, bh] -> (bh, win, i, pair, half)
        yc = yc.reshape(2, 64, qc, NWIN, BH)
        yc[:, :, 1::2] = yc[::-1, :, 1::2]     # odd pairs: swapped PSUM rows
        yc = yc.transpose(4, 3, 1, 2, 0)
        c[:, :, 2 * qc * cid:2 * qc * (cid + 1)] = (
            yc.reshape(BH, T, 2 * qc) * (1.0 / WSCALE))

    # block-0 boundary: the dropped window -1 holds v[0], v[1] at rows 62, 63
    _, Wprev = _build_filters(l_filter, r_filter)
    corr0 = (np.einsum("id,nd->nid", Wprev[62], vr[:, 0, :].astype(np.float64))
             + np.einsum("id,nd->nid", Wprev[63], vr[:, 1, :].astype(np.float64)))
    c[:, 0:64, :] += corr0.astype(np.float32)
    out = vr + c
    return out.reshape(B, H, T, D)


# revision 17
# speedup vs baseline: 1.5611x; 1.0153x over previous
"""DFSMN (order-9 IIR + 2-tap lookahead FIR along frames) on 8 Trainium2 cores.

Math: per (b, h, d) sequence the reference is an LTI filter along frames,
out = g_d * v (combined impulse response g, lags -2..inf, |g| below noise
past lag ~61).  The output is dominated by the identity tap (g[0] ~= 1,
all other taps ~0.05), so the kernel computes only the CORRECTION

    c = (G - I) v        (RMS(c) ~= 0.17 * RMS(out))

and the host adds back the exact fp32 v it already holds.  Every fp8
quantization error (x, w, y) then scales with RMS(c), not RMS(out):
end-to-end rel err ~= 8e-3 vs the 2e-2 gate, at HALF the bf16 traffic.

Blocking: 64-frame output blocks; block b needs lags -2..61 ->
windows b and b-1 of the (+2-shifted) input:

    c_blk[b] = Wcur_d^T @ win[b] + Wprev_d^T @ win[b-1]

with Wcur/Wprev [64x64] Toeplitz slices of g' = g - delta, scaled by 16
so the decaying tail clears the fp8 subnormal floor (host divides by 16).

Packing: 2 channels per 128 partitions (chA rows 0:64, chB rows 64:128),
PE quadrant tiling at (0,0)/(64,64) -> both channels' matmuls run
concurrently on the 128x128 array; windows x 64 (b,h) seqs ride the
free dim (16*64 = 1024 cols per channel-pair).

Per-core traffic (64 channels = 32 pairs): x 4.19MB + w 0.52MB +
y 4.19MB = 8.9MB fp8 -> ~25us HBM floor at 358 GB/s.

Window -1 of block 0 holds v[0:2] (rank-2): applied on the host in f64
together with nothing else -- the (G-I) form needs no other boundary fix.

Per-core tensors:
    x  [128, 32*1024] f8e4   col = q*1024 + win*64 + bh; part j<64 chA
                             frame j of win, j>=64 chB (ch = 2q, 2q+1)
    w  [128, 32*128]  f8e4   col = q*128 + {0:64 cur | 64:128 prev} lhsT
    y  [128, 32*1024] f8e4   col = q*1024 + blk*64 + bh; part i<64 chA
                             frame 64*blk+i, i>=64 chB; value = 16*c
"""

import numpy as np

import concourse.bass as bass
import concourse.bacc as bacc
import concourse.mybir as mybir
from concourse import tile
from concourse import bass_utils

B, H, T, D = 16, 4, 1024, 512
N_CORES = 8
DC = D // N_CORES          # 64 channels per core
PAIRS = DC // 2            # 32 channel pairs per core
BH = B * H                 # 64 sequences (matmul free dim)
NWIN = 16                  # 64-frame windows
FREE = NWIN * BH           # 1024 matmul free dim per pair
WCOL = 128                 # w cols per pair: cur [64] + prev [64]
LMAX = 61                  # last kept lag of the impulse response
WSCALE = 16.0              # host-side weight scale (psum = 16*c)
F32 = mybir.dt.float32
BF16 = mybir.dt.bfloat16
F8 = mybir.dt.float8e4

_NC_CACHE: dict = {}


def _build_nc(pairs: int = PAIRS):
    nc = bacc.Bacc("TRN2", target_bir_lowering=False, debug=False)
    x = nc.dram_tensor("x", [128, pairs * FREE], F8, kind="ExternalInput")
    w = nc.dram_tensor("w", [128, pairs * WCOL], F8, kind="ExternalInput")
    y = nc.dram_tensor("y", [128, pairs * FREE], F8, kind="ExternalOutput")
    xap, wap, yap = x.ap(), w.ap(), y.ap()
    # y-store groups (pairs): big in the middle, small last so the final
    # store's transfer+receipt tail is short
    YGROUPS = [(0, 8), (8, 8), (16, 8), (24, 6), (30, 2)]
    ymap = {}
    for s, n in YGROUPS:
        ymap[s + n - 1] = (s, n)       # trigger store on the group's last pair
    ystart = {s: (s, n) for s, n in YGROUPS}
    # ramped x load groups (pairs): small first tiles arrive early so the
    # matmul stream starts right after the PE warm-up burst
    GROUPS = [(0, 1), (1, 1), (2, 2), (4, 8), (12, 10), (22, 10)]
    gmap = {}                          # q -> (group_start, group_size)
    for s, n in GROUPS:
        for q in range(s, s + n):
            gmap[q] = (s, n)
    WGROUPS = [(0, 4), (4, 28)]

    with tile.TileContext(nc) as tc:
        with tc.tile_pool(name="xp", bufs=6) as xp, \
             tc.tile_pool(name="wp", bufs=len(WGROUPS)) as wp, \
             tc.tile_pool(name="op", bufs=3) as op, \
             tc.tile_pool(name="dp", bufs=1) as dp, \
             tc.tile_pool(name="pp", bufs=3, space="PSUM") as pp, \
             tc.tile_pool(name="pwp", bufs=1, space="PSUM") as pwp:
            # PE warm-up: HAM clock gate keeps PE at 1.2 GHz until ~3.4us of
            # sustained activity; burn the DMA-head time on dummy matmuls.
            # memset on DVE -- its preamble finishes ~3us before GpSimd's,
            # so the warm-up burst starts right after the framework barrier
            # instead of idling behind the first x DMA.
            dummy = dp.tile([128, 512], BF16, name="dummy")
            nc.vector.memset(dummy, 0.0)
            pw = pwp.tile([128, 512], F32, name="pw")
            for _ in range(4):
                nc.tensor.matmul(pw, lhsT=dummy[:, 0:128], rhs=dummy,
                                 start=True, stop=True)
            # w (0.52MB) loads early on the scalar HWDGE queue, stays resident
            wtiles = {}
            for s, n in WGROUPS:
                wt = wp.tile([128, n * WCOL], F8, name="wt")
                wsrc = wap.copy()
                wsrc.ap = wsrc.ap[:0] + [[pairs * WCOL, 128], [1, n * WCOL]]
                wsrc.offset = s * WCOL
                nc.scalar.dma_start(out=wt, in_=wsrc)
                wtiles[s] = wt
            wmap = {}
            for s, n in WGROUPS:
                for q in range(s, s + n):
                    wmap[q] = (s, n)
            xt = yt = None
            xbase = None
            for q in range(pairs):
                if gmap[q][0] == q:
                    s, n = gmap[q]
                    xt = xp.tile([128, n * FREE], F8, name="xt")
                    src = xap.copy()
                    src.ap = src.ap[:0] + [[pairs * FREE, 128], [1, n * FREE]]
                    src.offset = s * FREE
                    nc.sync.dma_start(out=xt, in_=src)
                    xbase = s
                if q in ystart:
                    ys, yn = ystart[q]
                    yt = op.tile([128, yn * FREE], F8, name="yt")
                xv = xt[:, (q - xbase) * FREE:(q - xbase + 1) * FREE]
                ws, _ = wmap[q]
                wt = wtiles[ws]
                wo = (q - ws) * WCOL
                ps = pp.tile([128, FREE], F32, name="ps")
                # quadrant-tiled matmuls: chA on (0,0), chB on (64,64) run
                # concurrently; cur covers all 16 blocks, prev covers blocks
                # 1..15 via the 64-col shift.  Block 0's prev-window term is
                # rank-2 in v[0:2] and applied on the host.  Cols 0:64 never
                # see a stop=True -> skip the sim's accumulation-group check.
                # Each matmul's PSUM output must fit one 2KB bank -> split
                # the 1024-col free dim into 512-col halves.  Issue order
                # strictly alternates the (0,0) / (64,64) PE quadrants so
                # every LDWEIGHTS targets a row group that differs from the
                # in-flight matmul's and gets pulled ahead (hidden).
                HB = FREE // 2
                # odd pairs swap their PSUM row halves (PE tiles (0,64) and
                # (64,0) instead of (0,0)/(64,64)) so consecutive pairs touch
                # disjoint array quadrants; the host unswaps when unpacking.
                swap = 64 if (q % 2) else 0
                for c0, c1, wd, st in (
                        (0, HB, 0, True),          # cur, half 0
                        (HB, FREE, 0, True),       # cur, half 1
                        (64, HB, 64, False),       # prev, half 0
                        (HB, FREE, 64, False)):    # prev, half 1
                    sh = 0 if st else 64
                    for base in (0, 64):
                        ob = base ^ swap
                        nc.tensor.matmul(
                            ps[ob:ob + 64, c0:c1],
                            lhsT=wt[base:base + 64, wo + wd:wo + wd + 64],
                            rhs=xv[base:base + 64, c0 - sh:c1 - sh],
                            start=st, stop=not st, skip_group_check=True)
                # alternate PSUM evacuation between DVE and ACT (GpSimd has
                # no PSUM port)
                ysl = yt[:, (q - ys) * FREE:(q - ys + 1) * FREE]
                if q % 2 == 0:
                    nc.vector.tensor_copy(ysl, ps)
                else:
                    nc.scalar.copy(ysl, ps)
                if q in ymap:
                    gs, gn = ymap[q]
                    dst = yap.copy()
                    dst.ap = dst.ap[:0] + [[pairs * FREE, 128], [1, gn * FREE]]
                    dst.offset = gs * FREE
                    nc.gpsimd.dma_start(out=dst, in_=yt)
                    # filler matmul bridges DMA-jitter stalls so the HAM
                    # clock gate never re-throttles mid-stream
                    if q < pairs - 1:
                        nc.tensor.matmul(pw, lhsT=dummy[:, 0:128],
                                         rhs=dummy, start=True, stop=True)
    nc.compile()
    return nc


def _get_nc(pairs: int = PAIRS):
    if pairs not in _NC_CACHE:
        _NC_CACHE[pairs] = _build_nc(pairs)
    return _NC_CACHE[pairs]


def _build_filters(l_filter: np.ndarray, r_filter: np.ndarray):
    """Returns Wcur, Wprev [64, 64, D] float64 lhsT Toeplitz blocks of the
    correction filter g' = g - delta (unscaled), truncated past lag LMAX."""
    c = l_filter[1:].astype(np.float64)            # (9, D) IIR coeffs
    d = c.shape[1]
    n_a = LMAX + 3
    a = np.zeros((n_a, d))
    a[0] = 1.0
    for n in range(1, n_a):
        for k in range(1, min(9, n) + 1):
            a[n] += c[k - 1] * a[n - k]
    q0 = 1.0 + l_filter[0].astype(np.float64)
    q1 = r_filter[0].astype(np.float64)
    q2 = r_filter[1].astype(np.float64)

    # gp[i] = correction tap at lag i-2, i in [0, LMAX+2]
    gp = np.zeros((LMAX + 3, d))
    gp[0] = q2 * a[0]
    gp[1] = q1 * a[0] + q2 * a[1]
    for lag in range(0, LMAX + 1):
        gp[lag + 2] = q0 * a[lag] + q1 * a[lag + 1] + q2 * a[lag + 2]
    gp[2] -= 1.0                                   # subtract identity

    jj = np.arange(64)[:, None]
    ii = np.arange(64)[None, :]
    lag_cur = ii - jj - 2                          # [-65, 61]
    lag_prev = ii - jj + 62                        # [-1, 125]
    Wcur = np.zeros((64, 64, d))
    Wprev = np.zeros((64, 64, d))
    mc = (lag_cur >= -2) & (lag_cur <= LMAX)
    mp = (lag_prev >= -2) & (lag_prev <= LMAX)
    Wcur[mc] = gp[(lag_cur + 2)[mc]]
    Wprev[mp] = gp[(lag_prev + 2)[mp]]
    return Wcur, Wprev


def _make_in_maps(v, l_filter, r_filter, n_cores=N_CORES):
    import ml_dtypes
    f8 = ml_dtypes.float8_e4m3
    Wcur, Wprev = _build_filters(l_filter, r_filter)
    # w lhsT layout: [part (half, j), pairall, col (cur|prev, i)]
    wsc = (Wcur * WSCALE).astype(np.float32)       # (64j, 64i, D)
    wsp = (Wprev * WSCALE).astype(np.float32)
    wall = np.empty((2, 64, D // 2, 2, 64), np.float32)
    # ch = 2*qall + half ; wall[half, j, qall, 0, i] = Wcur[j, i, ch]
    wall[0, :, :, 0, :] = wsc[:, :, 0::2].transpose(0, 2, 1)
    wall[0, :, :, 1, :] = wsp[:, :, 0::2].transpose(0, 2, 1)
    wall[1, :, :, 0, :] = wsc[:, :, 1::2].transpose(0, 2, 1)
    wall[1, :, :, 1, :] = wsp[:, :, 1::2].transpose(0, 2, 1)
    wall8 = wall.astype(f8)

    vr = np.asarray(v, dtype=np.float32).reshape(BH, T, D)
    vq = vr.astype(f8)                             # RNE quantize once
    # windows: frames 2..1025 (frames 1024,1025 zero) -> (BH, 16, 64, D)
    vpad = np.zeros((BH, NWIN * 64, D), f8)
    vpad[:, :T - 2] = vq[:, 2:, :]
    vw = vpad.reshape(BH, NWIN, 64, D)
    # xall[half, j, qall, win, bh] = vw[bh, win, j, 2*qall+half]
    xall = vw.transpose(3, 2, 1, 0).reshape(D // 2, 2, 64, NWIN, BH)
    xall = np.ascontiguousarray(xall.transpose(1, 2, 0, 3, 4))

    in_maps = []
    qc = PAIRS
    for cid in range(n_cores):
        sl = slice(cid * qc, (cid + 1) * qc)
        in_maps.append({
            "x": np.ascontiguousarray(xall[:, :, sl]).reshape(128, qc * FREE),
            "w": np.ascontiguousarray(wall8[:, :, sl]).reshape(128, qc * WCOL),
        })
    return in_maps


def kernel(v: np.ndarray, l_filter: np.ndarray, r_filter: np.ndarray,
           **_unused) -> np.ndarray:
    nc = _get_nc(PAIRS)
    in_maps = _make_in_maps(v, l_filter, r_filter)
    res = bass_utils.run_bass_kernel_spmd(nc, in_maps,
                                          core_ids=list(range(N_CORES)))
    vr = np.asarray(v, dtype=np.float32).reshape(BH, T, D)
    c = np.empty((BH, T, D), np.float32)
    qc = PAIRS
    for cid in range(N_CORES):
        yc = np.asarray(res.results[cid]["y"]).astype(np.float32)
        # [part (half, i), pair, win, bh] -> (bh, win, i, pair, half)
        yc = yc.reshape(2, 64, qc, NWIN, BH)
        yc[:, :, 1::2] = yc[::-1, :, 1::2]     # odd pairs: swapped PSUM rows
        yc = yc.transpose(4, 3, 1, 2, 0)
        c[:, :, 2 * qc * cid:2 * qc * (cid + 1)] = (
            yc.reshape(BH, T, 2 * qc) * (1.0 / WSCALE))

    # block-0 boundary: the dropped window -1 holds v[0], v[1] at rows 62, 63
    _, Wprev = _build_filters(l_filter, r_filter)
    corr0 = (np.einsum("id,nd->nid", Wprev[62], vr[:, 0, :].astype(np.float64))
             + np.einsum("id,nd->nid", Wprev[63], vr[:, 1, :].astype(np.float64)))
    c[:, 0:64, :] += corr0.astype(np.float32)
    out = vr + c
    return out.reshape(B, H, T, D)


# revision 18
# speedup vs baseline: 1.5872x; 1.0167x over previous
"""DFSMN (order-9 IIR + 2-tap lookahead FIR along frames) on 8 Trainium2 cores.

Math: per (b, h, d) sequence the reference is an LTI filter along frames,
out = g_d * v (combined impulse response g, lags -2..inf, |g| below noise
past lag ~61).  The output is dominated by the identity tap (g[0] ~= 1,
all other taps ~0.05), so the kernel computes only the CORRECTION

    c = (G - I) v        (RMS(c) ~= 0.17 * RMS(out))

and the host adds back the exact fp32 v it already holds.  Every fp8
quantization error (x, w, y) then scales with RMS(c), not RMS(out):
end-to-end rel err ~= 8e-3 vs the 2e-2 gate, at HALF the bf16 traffic.

Blocking: 64-frame output blocks; block b needs lags -2..61 ->
windows b and b-1 of the (+2-shifted) input:

    c_blk[b] = Wcur_d^T @ win[b] + Wprev_d^T @ win[b-1]

with Wcur/Wprev [64x64] Toeplitz slices of g' = g - delta, scaled by 16
so the decaying tail clears the fp8 subnormal floor (host divides by 16).

Packing: 2 channels per 128 partitions (chA rows 0:64, chB rows 64:128),
PE quadrant tiling at (0,0)/(64,64) -> both channels' matmuls run
concurrently on the 128x128 array; windows x 64 (b,h) seqs ride the
free dim (16*64 = 1024 cols per channel-pair).

Per-core traffic (64 channels = 32 pairs): x 4.19MB + w 0.52MB +
y 4.19MB = 8.9MB fp8 -> ~25us HBM floor at 358 GB/s.

Window -1 of block 0 holds v[0:2] (rank-2): applied on the host in f64
together with nothing else -- the (G-I) form needs no other boundary fix.

Per-core tensors:
    x  [128, 32*1024] f8e4   col = q*1024 + win*64 + bh; part j<64 chA
                             frame j of win, j>=64 chB (ch = 2q, 2q+1)
    w  [128, 32*128]  f8e4   col = q*128 + {0:64 cur | 64:128 prev} lhsT
    y  [128, 32*1024] f8e4   col = q*1024 + blk*64 + bh; part i<64 chA
                             frame 64*blk+i, i>=64 chB; value = 16*c
"""

import numpy as np

import concourse.bass as bass
import concourse.bacc as bacc
import concourse.mybir as mybir
from concourse import tile
from concourse import bass_utils

B, H, T, D = 16, 4, 1024, 512
N_CORES = 8
DC = D // N_CORES          # 64 channels per core
PAIRS = DC // 2            # 32 channel pairs per core
BH = B * H                 # 64 sequences (matmul free dim)
NWIN = 16                  # 64-frame windows
FREE = NWIN * BH           # 1024 matmul free dim per pair
WCOL = 128                 # w cols per pair: cur [64] + prev [64]
LMAX = 61                  # last kept lag of the impulse response
WSCALE = 16.0              # host-side weight scale (psum = 16*c)
F32 = mybir.dt.float32
BF16 = mybir.dt.bfloat16
F8 = mybir.dt.float8e4

_NC_CACHE: dict = {}


def _build_nc(pairs: int = PAIRS):
    nc = bacc.Bacc("TRN2", target_bir_lowering=False, debug=False)
    x = nc.dram_tensor("x", [128, pairs * FREE], F8, kind="ExternalInput")
    w = nc.dram_tensor("w", [128, pairs * WCOL], F8, kind="ExternalInput")
    y = nc.dram_tensor("y", [128, pairs * FREE], F8, kind="ExternalOutput")
    xap, wap, yap = x.ap(), w.ap(), y.ap()
    # y-store groups (pairs): big in the middle, small last so the final
    # store's transfer+receipt tail is short
    YGROUPS = [(0, 8), (8, 8), (16, 8), (24, 6), (30, 2)]
    ymap = {}
    for s, n in YGROUPS:
        ymap[s + n - 1] = (s, n)       # trigger store on the group's last pair
    ystart = {s: (s, n) for s, n in YGROUPS}
    # ramped x load groups (pairs): small first tiles arrive early so the
    # matmul stream starts right after the PE warm-up burst
    GROUPS = [(0, 1), (1, 1), (2, 2), (4, 8), (12, 10), (22, 10)]
    gmap = {}                          # q -> (group_start, group_size)
    for s, n in GROUPS:
        for q in range(s, s + n):
            gmap[q] = (s, n)
    WGROUPS = [(0, 4), (4, 28)]

    with tile.TileContext(nc) as tc:
        with tc.tile_pool(name="xp", bufs=6) as xp, \
             tc.tile_pool(name="wp", bufs=len(WGROUPS)) as wp, \
             tc.tile_pool(name="op", bufs=3) as op, \
             tc.tile_pool(name="dp", bufs=1) as dp, \
             tc.tile_pool(name="pp", bufs=3, space="PSUM") as pp, \
             tc.tile_pool(name="pwp", bufs=1, space="PSUM") as pwp:
            # PE warm-up: HAM clock gate keeps PE at 1.2 GHz until ~3.4us of
            # sustained activity; burn the DMA-head time on dummy matmuls.
            # memset on DVE -- its preamble finishes ~3us before GpSimd's,
            # so the warm-up burst starts right after the framework barrier
            # instead of idling behind the first x DMA.
            dummy = dp.tile([128, 512], BF16, name="dummy")
            nc.vector.memset(dummy, 0.0)
            pw = pwp.tile([128, 512], F32, name="pw")
            for _ in range(4):
                nc.tensor.matmul(pw, lhsT=dummy[:, 0:128], rhs=dummy,
                                 start=True, stop=True)
            # w (0.52MB) loads early on the scalar HWDGE queue, stays resident
            wtiles = {}
            for s, n in WGROUPS:
                wt = wp.tile([128, n * WCOL], F8, name="wt")
                wsrc = wap.copy()
                wsrc.ap = wsrc.ap[:0] + [[pairs * WCOL, 128], [1, n * WCOL]]
                wsrc.offset = s * WCOL
                nc.scalar.dma_start(out=wt, in_=wsrc)
                wtiles[s] = wt
            wmap = {}
            for s, n in WGROUPS:
                for q in range(s, s + n):
                    wmap[q] = (s, n)
            xt = yt = None
            xbase = None
            for q in range(pairs):
                if gmap[q][0] == q:
                    s, n = gmap[q]
                    xt = xp.tile([128, n * FREE], F8, name="xt")
                    src = xap.copy()
                    src.ap = src.ap[:0] + [[pairs * FREE, 128], [1, n * FREE]]
                    src.offset = s * FREE
                    nc.sync.dma_start(out=xt, in_=src)
                    xbase = s
                if q in ystart:
                    ys, yn = ystart[q]
                    yt = op.tile([128, yn * FREE], F8, name="yt")
                xv = xt[:, (q - xbase) * FREE:(q - xbase + 1) * FREE]
                ws, _ = wmap[q]
                wt = wtiles[ws]
                wo = (q - ws) * WCOL
                ps = pp.tile([128, FREE], F32, name="ps")
                # quadrant-tiled matmuls: chA on (0,0), chB on (64,64) run
                # concurrently; cur covers all 16 blocks, prev covers blocks
                # 1..15 via the 64-col shift.  Block 0's prev-window term is
                # rank-2 in v[0:2] and applied on the host.  Cols 0:64 never
                # see a stop=True -> skip the sim's accumulation-group check.
                # Each matmul's PSUM output must fit one 2KB bank -> split
                # the 1024-col free dim into 512-col halves.  Issue order
                # strictly alternates the (0,0) / (64,64) PE quadrants so
                # every LDWEIGHTS targets a row group that differs from the
                # in-flight matmul's and gets pulled ahead (hidden).
                HB = FREE // 2
                # odd pairs swap their PSUM row halves (PE tiles (0,64) and
                # (64,0) instead of (0,0)/(64,64)) so consecutive pairs touch
                # disjoint array quadrants; the host unswaps when unpacking.
                swap = 64 if (q % 2) else 0
                for c0, c1, wd, st in (
                        (0, HB, 0, True),          # cur, half 0
                        (HB, FREE, 0, True),       # cur, half 1
                        (64, HB, 64, False),       # prev, half 0
                        (HB, FREE, 64, False)):    # prev, half 1
                    sh = 0 if st else 64
                    for base in (0, 64):
                        ob = base ^ swap
                        nc.tensor.matmul(
                            ps[ob:ob + 64, c0:c1],
                            lhsT=wt[base:base + 64, wo + wd:wo + wd + 64],
                            rhs=xv[base:base + 64, c0 - sh:c1 - sh],
                            start=st, stop=not st, skip_group_check=True)
                # alternate PSUM evacuation between DVE and ACT (GpSimd has
                # no PSUM port)
                ysl = yt[:, (q - ys) * FREE:(q - ys + 1) * FREE]
                if q % 2 == 0:
                    nc.vector.tensor_copy(ysl, ps)
                else:
                    nc.scalar.copy(ysl, ps)
                if q in ymap:
                    gs, gn = ymap[q]
                    dst = yap.copy()
                    dst.ap = dst.ap[:0] + [[pairs * FREE, 128], [1, gn * FREE]]
                    dst.offset = gs * FREE
                    # ACT's HWDGE ring (separate from Sync's, which carries
                    # the x loads): faster issue than SWDGE and no slow
                    # gpsimd descriptor-ring drain in the postamble
                    nc.scalar.dma_start(out=dst, in_=yt)
                    # filler matmul bridges DMA-jitter stalls so the HAM
                    # clock gate never re-throttles mid-stream
                    if q < pairs - 1:
                        nc.tensor.matmul(pw, lhsT=dummy[:, 0:128],
                                         rhs=dummy, start=True, stop=True)
    nc.compile()
    return nc


def _get_nc(pairs: int = PAIRS):
    if pairs not in _NC_CACHE:
        _NC_CACHE[pairs] = _build_nc(pairs)
    return _NC_CACHE[pairs]


def _build_filters(l_filter: np.ndarray, r_filter: np.ndarray):
    """Returns Wcur, Wprev [64, 64, D] float64 lhsT Toeplitz blocks of the
    correction filter g' = g - delta (unscaled), truncated past lag LMAX."""
    c = l_filter[1:].astype(np.float64)            # (9, D) IIR coeffs
    d = c.shape[1]
    n_a = LMAX + 3
    a = np.zeros((n_a, d))
    a[0] = 1.0
    for n in range(1, n_a):
        for k in range(1, min(9, n) + 1):
            a[n] += c[k - 1] * a[n - k]
    q0 = 1.0 + l_filter[0].astype(np.float64)
    q1 = r_filter[0].astype(np.float64)
    q2 = r_filter[1].astype(np.float64)

    # gp[i] = correction tap at lag i-2, i in [0, LMAX+2]
    gp = np.zeros((LMAX + 3, d))
    gp[0] = q2 * a[0]
    gp[1] = q1 * a[0] + q2 * a[1]
    for lag in range(0, LMAX + 1):
        gp[lag + 2] = q0 * a[lag] + q1 * a[lag + 1] + q2 * a[lag + 2]
    gp[2] -= 1.0                                   # subtract identity

    jj = np.arange(64)[:, None]
    ii = np.arange(64)[None, :]
    lag_cur = ii - jj - 2                          # [-65, 61]
    lag_prev = ii - jj + 62                        # [-1, 125]
    Wcur = np.zeros((64, 64, d))
    Wprev = np.zeros((64, 64, d))
    mc = (lag_cur >= -2) & (lag_cur <= LMAX)
    mp = (lag_prev >= -2) & (lag_prev <= LMAX)
    Wcur[mc] = gp[(lag_cur + 2)[mc]]
    Wprev[mp] = gp[(lag_prev + 2)[mp]]
    return Wcur, Wprev


def _make_in_maps(v, l_filter, r_filter, n_cores=N_CORES):
    import ml_dtypes
    f8 = ml_dtypes.float8_e4m3
    Wcur, Wprev = _build_filters(l_filter, r_filter)
    # w lhsT layout: [part (half, j), pairall, col (cur|prev, i)]
    wsc = (Wcur * WSCALE).astype(np.float32)       # (64j, 64i, D)
    wsp = (Wprev * WSCALE).astype(np.float32)
    wall = np.empty((2, 64, D // 2, 2, 64), np.float32)
    # ch = 2*qall + half ; wall[half, j, qall, 0, i] = Wcur[j, i, ch]
    wall[0, :, :, 0, :] = wsc[:, :, 0::2].transpose(0, 2, 1)
    wall[0, :, :, 1, :] = wsp[:, :, 0::2].transpose(0, 2, 1)
    wall[1, :, :, 0, :] = wsc[:, :, 1::2].transpose(0, 2, 1)
    wall[1, :, :, 1, :] = wsp[:, :, 1::2].transpose(0, 2, 1)
    wall8 = wall.astype(f8)

    vr = np.asarray(v, dtype=np.float32).reshape(BH, T, D)
    vq = vr.astype(f8)                             # RNE quantize once
    # windows: frames 2..1025 (frames 1024,1025 zero) -> (BH, 16, 64, D)
    vpad = np.zeros((BH, NWIN * 64, D), f8)
    vpad[:, :T - 2] = vq[:, 2:, :]
    vw = vpad.reshape(BH, NWIN, 64, D)
    # xall[half, j, qall, win, bh] = vw[bh, win, j, 2*qall+half]
    xall = vw.transpose(3, 2, 1, 0).reshape(D // 2, 2, 64, NWIN, BH)
    xall = np.ascontiguousarray(xall.transpose(1, 2, 0, 3, 4))

    in_maps = []
    qc = PAIRS
    for cid in range(n_cores):
        sl = slice(cid * qc, (cid + 1) * qc)
        in_maps.append({
            "x": np.ascontiguousarray(xall[:, :, sl]).reshape(128, qc * FREE),
            "w": np.ascontiguousarray(wall8[:, :, sl]).reshape(128, qc * WCOL),
        })
    return in_maps


def kernel(v: np.ndarray, l_filter: np.ndarray, r_filter: np.ndarray,
           **_unused) -> np.ndarray:
    nc = _get_nc(PAIRS)
    in_maps = _make_in_maps(v, l_filter, r_filter)
    res = bass_utils.run_bass_kernel_spmd(nc, in_maps,
                                          core_ids=list(range(N_CORES)))
    vr = np.asarray(v, dtype=np.float32).reshape(BH, T, D)
    c = np.empty((BH, T, D), np.float32)
    qc = PAIRS
    for cid in range(N_CORES):
        yc = np.asarray(res.results[cid]["y"]).astype(np.float32)
        # [part (half, i), pair, win, bh] -> (bh, win, i, pair, half)
        yc = yc.reshape(2, 64, qc, NWIN, BH)
        yc[:, :, 1::2] = yc[::-1, :, 1::2]     # odd pairs: swapped PSUM rows
        yc = yc.transpose(4, 3, 1, 2, 0)
        c[:, :, 2 * qc * cid:2 * qc * (cid + 1)] = (
            yc.reshape(BH, T, 2 * qc) * (1.0 / WSCALE))

    # block-0 boundary: the dropped window -1 holds v[0], v[1] at rows 62, 63
    _, Wprev = _build_filters(l_filter, r_filter)
    corr0 = (np.einsum("id,nd->nid", Wprev[62], vr[:, 0, :].astype(np.float64))
             + np.einsum("id,nd->nid", Wprev[63], vr[:, 1, :].astype(np.float64)))
    c[:, 0:64, :] += corr0.astype(np.float32)
    out = vr + c
    return out.reshape(B, H, T, D)
